# revision 8
# baseline (speedup 1.0000x reference)
"""Trainium2 Bass kernel for nn_ExpertFFNEnsemble (MoE routing, 8 experts, top-2).

Strategy: data-parallel over tokens (8192 tokens -> 1024/core, 8 cores).
Per core, fully on device:
  router (fp32)  -> top-2 + combine weights (sigmoid of logit gap)
  compaction     -> per-expert capacity buckets via cumsum-ranks + indirect scatter
  dispatch       -> one dma_gather(transpose=True) of all bucket rows (bf16)
  expert FFNs    -> bf16 matmuls, fp32 PSUM, exact-gelu ACT epilogue
  shared expert  -> dense bf16 FFN over the core's 1024 tokens
  combine + LN   -> indirect row gathers of the two bucket outputs + LayerNorm
No cross-core communication; host only shards tokens / casts weights to bf16
and concatenates the per-core output slices.
"""

import sys

sys.path.insert(0, "/opt/trn_rl_repo")

import numpy as np
import ml_dtypes

import concourse.bass as bass
import concourse.mybir as mybir
import concourse.tile as tile
from concourse import bacc
from concourse.bass import IndirectOffsetOnAxis
from concourse.bass_utils import run_bass_kernel_spmd

P = 128
B, S, D, F = 4, 2048, 1024, 4096
F2 = F // 2
E = 8
NCORES = 8
T = (B * S) // NCORES           # 1024 tokens per core
NT = T // P                     # 8 token tiles
ND = D // P                     # 8 d-chunks
NF = F // P                     # 32 f-chunks
NF2 = F2 // P                   # 16 f2-chunks
CAP = 384                       # per-expert token capacity (3 x 128)
NIDX = E * CAP                  # 2560 bucket rows (%128 == 0)
TRASH = NIDX                    # overflow-redirect row
BID_ROWS = NIDX + P             # bucket-id rows incl. trash region
YBK_ROWS = NIDX + P             # fc2 output rows incl. trash region
LN_EPS = 1e-5
FT = mybir.ActivationFunctionType
dt = mybir.dt
AX = mybir.AxisListType
OP = mybir.AluOpType

_PROGRAM = None


def _cap_tiles():
    """Token tiles within one expert bucket: list of (row_offset, rows)."""
    out = []
    r = 0
    while r < CAP:
        n = min(P, CAP - r)
        out.append((r, n))
        r += n
    return out


def build_program():
    nc = bacc.Bacc("TRN2", target_bir_lowering=False, debug=False,
                   num_devices=NCORES)

    def din(name, shape, dtype):
        return nc.dram_tensor(name, list(shape), dtype, kind="ExternalInput").ap()

    x_ap = din("x", [T, D], dt.float32)
    rw_ap = din("rw", [D, E], dt.float32)
    rb_ap = din("rb", [1, E], dt.float32)
    w1_ap = din("w1", [E, D, F], dt.bfloat16)
    w2_ap = din("w2", [E, F, D], dt.bfloat16)
    b1_ap = din("b1", [E, F], dt.float32)
    gate_ap = din("gate", [E, F], dt.float32)
    b2_ap = din("b2", [E, D], dt.bfloat16)
    sw1_ap = din("sw1", [D, F2], dt.bfloat16)
    sb1_ap = din("sb1", [F2], dt.float32)
    sgate_ap = din("sgate", [F2], dt.float32)
    sw2_ap = din("sw2", [F2, D], dt.bfloat16)
    sb2_ap = din("sb2", [1, D], dt.bfloat16)
    shw_ap = din("shw", [1, 1], dt.float32)
    lng_ap = din("lng", [1, D], dt.float32)
    lnb_ap = din("lnb", [1, D], dt.float32)
    # host-provided constants
    iota8_ap = din("iota8", [P, 8], dt.float32)
    iotat_ap = din("iotat", [T, 1], dt.int16)
    tri_ap = din("tri", [P, P], dt.bfloat16)
    ident_ap = din("ident", [P, P], dt.float32)
    onesb_ap = din("onesb", [1, P], dt.bfloat16)
    onesf_ap = din("onesf", [1, P], dt.float32)
    idw_id_ap = din("idw_id", [16, T // 16], dt.int16)  # wrapped identity idx

    out_ap = nc.dram_tensor("out", [T, D], dt.float32, kind="ExternalOutput").ap()

    xbf_dram = nc.dram_tensor("xbf_i", [T, D], dt.bfloat16).ap()
    bid_dram = nc.dram_tensor("bid_i", [BID_ROWS, 1], dt.int16).ap()
    ybk_dram = nc.dram_tensor("ybk_i", [YBK_ROWS, D], dt.float32).ap()

    with tile.TileContext(nc) as tc:
        with (
            tc.tile_pool(name="persist", bufs=1) as pp,
            tc.tile_pool(name="small", bufs=2) as sp,
        ):
            # ---- constants ----
            iota8 = pp.tile([P, 8], dt.float32)
            nc.sync.dma_start(iota8[:], iota8_ap[:])
            tri = pp.tile([P, P], dt.bfloat16)
            nc.sync.dma_start(tri[:], tri_ap[:])
            ident = pp.tile([P, P], dt.float32)
            nc.sync.dma_start(ident[:], ident_ap[:])
            onesb = pp.tile([1, P], dt.bfloat16)
            nc.sync.dma_start(onesb[:], onesb_ap[:])
            onesf = pp.tile([1, P], dt.float32)
            nc.sync.dma_start(onesf[:], onesf_ap[:])
            rw_sb = pp.tile([P, ND, E], dt.float32)
            nc.sync.dma_start(rw_sb[:], rw_ap.rearrange("(k p) e -> p k e", p=P))
            rb_sb = pp.tile([1, E], dt.float32)
            nc.sync.dma_start(rb_sb[:], rb_ap[:, :])
            eps_t = pp.tile([P, 1], dt.float32)
            nc.vector.memset(eps_t[:], LN_EPS)

            # gate / b1*gate per expert: [128, E, NF]
            gate_sb = pp.tile([P, E, NF], dt.float32)
            nc.sync.dma_start(gate_sb[:], gate_ap.rearrange("e (m p) -> p e m", p=P))
            b1_sb = pp.tile([P, E, NF], dt.float32)
            nc.sync.dma_start(b1_sb[:], b1_ap.rearrange("e (m p) -> p e m", p=P))
            b1g_sb = pp.tile([P, E, NF], dt.float32)
            nc.vector.tensor_mul(b1g_sb[:], b1_sb[:], gate_sb[:])
            b2_sb = pp.tile([1, E, D], dt.bfloat16)
            nc.sync.dma_start(b2_sb[:], b2_ap.rearrange("e d2 -> e d2")[None, :, :])
            sg_sb = pp.tile([P, NF2], dt.float32)
            nc.sync.dma_start(sg_sb[:], sgate_ap.rearrange("(m p) -> p m", p=P))
            sb1_sb = pp.tile([P, NF2], dt.float32)
            nc.sync.dma_start(sb1_sb[:], sb1_ap.rearrange("(m p) -> p m", p=P))
            sb1g_sb = pp.tile([P, NF2], dt.float32)
            nc.vector.tensor_mul(sb1g_sb[:], sb1_sb[:], sg_sb[:])
            sb2_sb = pp.tile([1, D], dt.bfloat16)
            nc.sync.dma_start(sb2_sb[:], sb2_ap[:, :])
            lng_bc = pp.tile([P, D], dt.float32)
            nc.sync.dma_start(lng_bc[:], lng_ap.to_broadcast([P, D]))
            lnb_bc = pp.tile([P, D], dt.float32)
            nc.sync.dma_start(lnb_bc[:], lnb_ap.to_broadcast([P, D]))

            # sigmoid(shared_weight) broadcast to [128,1] via ones-column matmul
            shw_sb = sp.tile([1, 1], dt.float32, tag="shw")
            nc.sync.dma_start(shw_sb[:], shw_ap[:, :])
            sig1 = sp.tile([1, 1], dt.float32, tag="sig1")
            nc.scalar.activation(sig1[:], shw_sb[:], FT.Sigmoid)
            sig_bc = pp.tile([P, 1], dt.float32)

            # zero tile for pre-clearing internal DRAM
            zid = sp.tile([P, BID_ROWS // P], dt.int16, tag="zid")
            nc.vector.memset(zid[:], 0)
            nc.sync.dma_start(
                bid_dram.rearrange("(p c) one -> p (c one)", p=P), zid[:])
            ztrash = sp.tile([P, D], dt.float32, tag="ztrash")
            nc.vector.memset(ztrash[:], 0.0)
            nc.sync.dma_start(ybk_dram[TRASH:TRASH + P, :], ztrash[:])

            # per-token routing results, kept for the combine phase
            cw_tiles = []
            pos_tiles = []

            # xT bf16, chunk-major: [128, NT, ND, P]
            xTb = pp.tile([P, NT, ND, P], dt.bfloat16)

            with (
                tc.tile_pool(name="ph0", bufs=3) as p0,
                tc.tile_pool(name="ph0ps", bufs=2, space="PSUM") as p0ps,
                tc.tile_pool(name="xTf_pool", bufs=1) as pxT,
            ):
                psig = p0ps.tile([P, 8], dt.float32, tag="pl", name="psig",
                                 space="PSUM")
                nc.tensor.matmul(psig[:, 0:1], lhsT=onesf[:, :], rhs=sig1[:, :],
                                 start=True, stop=True)
                nc.vector.tensor_copy(sig_bc[:], psig[:, 0:1])

                # ---- phase 0: stage xbf (bf16) + xTf (fp32 transpose) ----
                xTf = pxT.tile([P, ND, T], dt.float32)
                for i in range(NT):
                    xt = p0.tile([P, D], dt.float32, tag="xt")
                    nc.sync.dma_start(xt[:], x_ap[i * P:(i + 1) * P, :])
                    xb = p0.tile([P, D], dt.bfloat16, tag="xb")
                    nc.vector.tensor_copy(xb[:], xt[:])
                    nc.sync.dma_start(xbf_dram[i * P:(i + 1) * P, :], xb[:])
                    for k in range(ND):
                        ptr = p0ps.tile([P, P], dt.float32, tag="ptr", space="PSUM")
                        nc.tensor.transpose(
                            ptr[:], xt[:, k * P:(k + 1) * P], ident[:])
                        nc.vector.tensor_copy(
                            xTf[:, k, i * P:(i + 1) * P], ptr[:])

                # xTb via identity dma_gather (bf16, transposed)
                idw_id = p0.tile([P, T // 16], dt.int16, tag="idw_id")
                for g in range(8):
                    nc.sync.dma_start(idw_id[g * 16:(g + 1) * 16, :], idw_id_ap[:, :])
                for j in range(T // P):
                    nc.gpsimd.dma_gather(
                        out_ap=xTb[:, j, :, :],
                        in_ap=xbf_dram[:, :],
                        idxs_ap=idw_id[:, j * 8:(j + 1) * 8],
                        num_idxs=P, num_idxs_reg=P, elem_size=D,
                        transpose=True)

                # ---- phase 1: router + compaction ----
                carry = pp.tile([E, 1], dt.float32)
                nc.vector.memset(carry[:], 0.0)
                rank_sb = pp.tile([E, T], dt.float32)

                for i in range(NT):
                    tsl = slice(i * P, (i + 1) * P)
                    pl = p0ps.tile([P, 8], dt.float32, tag="pl", space="PSUM")
                    for k in range(ND):
                        nc.tensor.matmul(pl[:], lhsT=xTf[:, k, tsl],
                                         rhs=rw_sb[:, k, :],
                                         start=(k == 0), stop=False)
                    nc.tensor.matmul(pl[:], lhsT=onesf[:, :], rhs=rb_sb[:, :],
                                     start=False, stop=True)
                    vals = p0.tile([P, 8], dt.float32, tag="vals")
                    idx = p0.tile([P, 8], dt.uint32, tag="idx")
                    lt = p0.tile([P, 8], dt.float32, tag="lt")
                    nc.vector.tensor_copy(lt[:], pl[:])
                    nc.vector.max_with_indices(vals[:], idx[:], lt[:])

                    d01 = p0.tile([P, 1], dt.float32, tag="d01")
                    nc.vector.tensor_sub(d01[:], vals[:, 0:1], vals[:, 1:2])
                    cw = pp.tile([P, 2], dt.float32, tag=f"cw{i}")
                    nc.scalar.activation(cw[:, 0:1], d01[:], FT.Sigmoid)
                    nc.scalar.activation(cw[:, 1:2], d01[:], FT.Sigmoid, scale=-1.0)
                    cw_tiles.append(cw)

                    ef = p0.tile([P, 2], dt.float32, tag="ef")
                    nc.vector.tensor_copy(ef[:], idx[:, 0:2])
                    oh0 = p0.tile([P, 8], dt.float32, tag="oh0")
                    oh1 = p0.tile([P, 8], dt.float32, tag="oh1")
                    nc.vector.tensor_tensor(
                        out=oh0[:], in0=ef[:, 0:1].to_broadcast([P, 8]),
                        in1=iota8[:], op=OP.is_equal)
                    nc.vector.tensor_tensor(
                        out=oh1[:], in0=ef[:, 1:2].to_broadcast([P, 8]),
                        in1=iota8[:], op=OP.is_equal)
                    A = p0.tile([P, 8], dt.bfloat16, tag="A")
                    nc.vector.tensor_add(A[:], oh0[:], oh1[:])

                    pr = p0ps.tile([E, P], dt.float32, tag="pr", space="PSUM")
                    nc.tensor.matmul(pr[:], lhsT=A[:], rhs=tri[:],
                                     start=True, stop=True)
                    nc.vector.tensor_scalar_add(rank_sb[:, tsl], pr[:],
                                                carry[:, 0:1])
                    nc.vector.tensor_copy(
                        carry[:], rank_sb[:, i * P + P - 1:i * P + P])

                    prt = p0ps.tile([P, E], dt.float32, tag="prt", space="PSUM")
                    nc.tensor.transpose(prt[:], rank_sb[:, tsl], ident[:E, :E])
                    rank_t = p0.tile([P, E], dt.float32, tag="rank_t")
                    nc.vector.tensor_copy(rank_t[:], prt[:])

                    tmp = p0.tile([P, 8], dt.float32, tag="tmp")
                    r0 = p0.tile([P, 1], dt.float32, tag="r0")
                    r1 = p0.tile([P, 1], dt.float32, tag="r1")
                    nc.vector.tensor_mul(tmp[:], oh0[:], rank_t[:])
                    nc.vector.reduce_sum(r0[:], tmp[:], axis=AX.X)
                    nc.vector.tensor_mul(tmp[:], oh1[:], rank_t[:])
                    nc.vector.reduce_sum(r1[:], tmp[:], axis=AX.X)

                    posf = p0.tile([P, 2], dt.float32, tag="posf")
                    nc.vector.tensor_scalar(out=posf[:, 0:1], in0=ef[:, 0:1],
                                            scalar1=float(CAP), scalar2=None,
                                            op0=OP.mult)
                    nc.vector.tensor_scalar(out=posf[:, 1:2], in0=ef[:, 1:2],
                                            scalar1=float(CAP), scalar2=None,
                                            op0=OP.mult)
                    nc.vector.scalar_tensor_tensor(
                        out=posf[:, 0:1], in0=r0[:], scalar=-1.0,
                        in1=posf[:, 0:1], op0=OP.add, op1=OP.add)
                    nc.vector.scalar_tensor_tensor(
                        out=posf[:, 1:2], in0=r1[:], scalar=-1.0,
                        in1=posf[:, 1:2], op0=OP.add, op1=OP.add)
                    ovf = p0.tile([P, 2], dt.uint8, tag="ovf")
                    nc.vector.tensor_scalar(out=ovf[:, 0:1], in0=r0[:],
                                            scalar1=float(CAP), scalar2=None,
                                            op0=OP.is_gt)
                    nc.vector.tensor_scalar(out=ovf[:, 1:2], in0=r1[:],
                                            scalar1=float(CAP), scalar2=None,
                                            op0=OP.is_gt)
                    trash = p0.tile([P, 2], dt.float32, tag="trash")
                    nc.vector.memset(trash[:], float(TRASH))
                    nc.vector.copy_predicated(posf[:], ovf[:], trash[:])
                    pos_i = pp.tile([P, 2], dt.int32, tag=f"pos{i}")
                    nc.vector.tensor_copy(pos_i[:], posf[:])
                    pos_tiles.append(pos_i)

                    tok16 = p0.tile([P, 1], dt.int16, tag="tok16")
                    nc.sync.dma_start(tok16[:], iotat_ap[tsl, :])
                    for s in range(2):
                        nc.gpsimd.indirect_dma_start(
                            out=bid_dram[:, :],
                            out_offset=IndirectOffsetOnAxis(
                                ap=pos_i[:, s:s + 1], axis=0),
                            in_=tok16[:, :], in_offset=None)

            # ---- phase 2: gather all bucket rows (transposed bf16) ----
            with tc.tile_pool(name="gx_pool", bufs=1) as pgx:
                gxT = pgx.tile([P, NIDX // P, ND, P], dt.bfloat16)
                with tc.tile_pool(name="idxw_pool", bufs=1) as pidx:
                    idxw = pidx.tile([P, NIDX // 16], dt.int16)
                    for g in range(8):
                        nc.sync.dma_start(
                            idxw[g * 16:(g + 1) * 16, :],
                            bid_dram[:NIDX, :].rearrange(
                                "(c p) one -> p (c one)", p=16))
                    for j in range(NIDX // P):
                        nc.gpsimd.dma_gather(
                            out_ap=gxT[:, j, :, :],
                            in_ap=xbf_dram[:, :],
                            idxs_ap=idxw[:, j * 8:(j + 1) * 8],
                            num_idxs=P, num_idxs_reg=P, elem_size=D,
                            transpose=True)

                # ---- phase 3: expert FFNs ----
                with (
                    tc.tile_pool(name="w1p", bufs=3) as w1p,
                    tc.tile_pool(name="w2p", bufs=3) as w2p,
                    tc.tile_pool(name="hTp", bufs=1) as hTp,
                    tc.tile_pool(name="ps1", bufs=2, space="PSUM") as ps1,
                    tc.tile_pool(name="ps2", bufs=1, space="PSUM") as ps2,
                ):
                    for e in range(E):
                        hT = hTp.tile([P, NF, CAP], dt.bfloat16, tag="hT")
                        # fc1 over 512-wide F chunks
                        for m5 in range(F // 512):
                            w1m = w1p.tile([P, ND, 512], dt.bfloat16, tag="w1m")
                            nc.sync.dma_start(
                                w1m[:],
                                w1_ap[e].rearrange("(k p) f -> p k f", p=P)
                                [:, :, m5 * 512:(m5 + 1) * 512])
                            for mm in range(4):
                                m = m5 * 4 + mm
                                pm = ps1.tile([P, CAP], dt.float32, tag="pm",
                                              space="PSUM")
                                for k in range(ND):
                                    nc.tensor.matmul(
                                        pm[:],
                                        lhsT=w1m[:, k, mm * P:(mm + 1) * P],
                                        rhs=gxT[:, 3 * e:3 * e + 3, k, :],
                                        start=(k == 0), stop=(k == ND - 1))
                                nc.scalar.activation(
                                    hT[:, m, :], pm[:], FT.Gelu,
                                    bias=b1g_sb[:, e, m:m + 1],
                                    scale=gate_sb[:, e, m:m + 1])
                        # fc2: 6 psum tiles held across the k loop
                        pys = [
                            ps2.tile([P, 512], dt.float32, tag=f"py{j}_{n}",
                                     name=f"py_e{e}_{j}_{n}", space="PSUM")
                            for j, (ro, rn) in enumerate(_cap_tiles())
                            for n in range(2)
                        ]
                        for k in range(NF):
                            w2k = w2p.tile([P, D], dt.bfloat16, tag="w2k")
                            nc.sync.dma_start(
                                w2k[:],
                                w2_ap[e, k * P:(k + 1) * P, :])
                            pi = 0
                            for (ro, rn) in _cap_tiles():
                                for n in range(2):
                                    nc.tensor.matmul(
                                        pys[pi][:rn, :],
                                        lhsT=hT[:, k, ro:ro + rn],
                                        rhs=w2k[:, n * 512:(n + 1) * 512],
                                        start=(k == 0), stop=False)
                                    pi += 1
                        pi = 0
                        for (ro, rn) in _cap_tiles():
                            for n in range(2):
                                nc.tensor.matmul(
                                    pys[pi][:rn, :], lhsT=onesb[:, :rn],
                                    rhs=b2_sb[:, e, n * 512:(n + 1) * 512],
                                    start=False, stop=True)
                                yev = w2p.tile([P, 512], dt.float32, tag="yev",
                                               name=f"yev_{e}_{pi}")
                                nc.vector.tensor_copy(yev[:rn, :], pys[pi][:rn, :])
                                nc.sync.dma_start(
                                    ybk_dram[e * CAP + ro:e * CAP + ro + rn,
                                             n * 512:(n + 1) * 512],
                                    yev[:rn, :])
                                pi += 1

            # ---- phase 4: shared expert ----
            ys = pp.tile([P, NT, D], dt.float32)
            with (
                tc.tile_pool(name="sw1p", bufs=3) as sw1p,
                tc.tile_pool(name="sw2p", bufs=1) as sw2p,
                tc.tile_pool(name="hsTp", bufs=1) as hsTp,
                tc.tile_pool(name="ps3", bufs=2, space="PSUM") as ps3,
                tc.tile_pool(name="ps4", bufs=4, space="PSUM") as ps4,
            ):
                hsT = hsTp.tile([P, NF2, T], dt.bfloat16)
                for m5 in range(F2 // 512):
                    sw1m = sw1p.tile([P, ND, 512], dt.bfloat16, tag="sw1m")
                    nc.sync.dma_start(
                        sw1m[:],
                        sw1_ap.rearrange("(k p) f -> p k f", p=P)
                        [:, :, m5 * 512:(m5 + 1) * 512])
                    for mm in range(4):
                        m = m5 * 4 + mm
                        for n in range(2):
                            pm = ps3.tile([P, 512], dt.float32, tag="pm3",
                                          space="PSUM")
                            for k in range(ND):
                                nc.tensor.matmul(
                                    pm[:],
                                    lhsT=sw1m[:, k, mm * P:(mm + 1) * P],
                                    rhs=xTb[:, 4 * n:4 * n + 4, k, :],
                                    start=(k == 0), stop=(k == ND - 1))
                            nc.scalar.activation(
                                hsT[:, m, n * 512:(n + 1) * 512], pm[:],
                                FT.Gelu, bias=sb1g_sb[:, m:m + 1],
                                scale=sg_sb[:, m:m + 1])

                sw2_sb = sw2p.tile([P, NF2, D], dt.bfloat16)
                nc.sync.dma_start(
                    sw2_sb[:], sw2_ap.rearrange("(k p) d2 -> p k d2", p=P))
                for j in range(NT):
                    jsl = slice(j * P, (j + 1) * P)
                    for n in range(2):
                        pyt = ps4.tile([P, 512], dt.float32, tag="py4",
                                       space="PSUM")
                        for k in range(NF2):
                            nc.tensor.matmul(
                                pyt[:], lhsT=hsT[:, k, jsl],
                                rhs=sw2_sb[:, k, n * 512:(n + 1) * 512],
                                start=(k == 0), stop=False)
                        nc.tensor.matmul(
                            pyt[:], lhsT=onesb[:, :],
                            rhs=sb2_sb[:, n * 512:(n + 1) * 512],
                            start=False, stop=True)
                        # ys = sigmoid(shared_weight) * (fc2s + sb2)
                        nc.scalar.activation(
                            ys[:, j, n * 512:(n + 1) * 512], pyt[:],
                            FT.Copy, scale=sig_bc[:, 0:1])

            # ---- phase 5: combine + LayerNorm ----
            with tc.tile_pool(name="ph5", bufs=3) as p5:
                for i in range(NT):
                    g0 = p5.tile([P, D], dt.float32, tag="g0")
                    g1 = p5.tile([P, D], dt.float32, tag="g1")
                    nc.gpsimd.indirect_dma_start(
                        out=g0[:], out_offset=None, in_=ybk_dram[:, :],
                        in_offset=IndirectOffsetOnAxis(
                            ap=pos_tiles[i][:, 0:1], axis=0))
                    nc.gpsimd.indirect_dma_start(
                        out=g1[:], out_offset=None, in_=ybk_dram[:, :],
                        in_offset=IndirectOffsetOnAxis(
                            ap=pos_tiles[i][:, 1:2], axis=0))
                    comb = p5.tile([P, D], dt.float32, tag="comb")
                    nc.vector.scalar_tensor_tensor(
                        out=comb[:], in0=g0[:], scalar=cw_tiles[i][:, 0:1],
                        in1=ys[:, i, :], op0=OP.mult, op1=OP.add)
                    nc.vector.scalar_tensor_tensor(
                        out=comb[:], in0=g1[:], scalar=cw_tiles[i][:, 1:2],
                        in1=comb[:], op0=OP.mult, op1=OP.add)
                    mu = p5.tile([P, 1], dt.float32, tag="mu")
                    nc.vector.reduce_sum(mu[:], comb[:], axis=AX.X)
                    nc.vector.tensor_scalar_mul(mu[:], mu[:], 1.0 / D)
                    yc = p5.tile([P, D], dt.float32, tag="yc")
                    nc.vector.tensor_scalar(out=yc[:], in0=comb[:],
                                            scalar1=mu[:, 0:1], scalar2=None,
                                            op0=OP.subtract)
                    sq = p5.tile([P, D], dt.float32, tag="sq")
                    varsum = p5.tile([P, 1], dt.float32, tag="varsum")
                    nc.scalar.activation(sq[:], yc[:], FT.Square,
                                         accum_out=varsum[:])
                    sd = p5.tile([P, 1], dt.float32, tag="sd")
                    nc.scalar.activation(sd[:], varsum[:], FT.Sqrt,
                                         scale=1.0 / D, bias=eps_t[:, 0:1])
                    rinv = p5.tile([P, 1], dt.float32, tag="rinv")
                    nc.vector.reciprocal(rinv[:], sd[:])
                    o1 = p5.tile([P, D], dt.float32, tag="o1")
                    nc.vector.scalar_tensor_tensor(
                        out=o1[:], in0=yc[:], scalar=rinv[:, 0:1],
                        in1=lng_bc[:], op0=OP.mult, op1=OP.mult)
                    nc.vector.tensor_add(o1[:], o1[:], lnb_bc[:])
                    nc.sync.dma_start(out_ap[i * P:(i + 1) * P, :], o1[:])

    nc.compile()
    return nc


def _consts():
    iota8 = np.tile(np.arange(8, dtype=np.float32), (P, 1))
    iotat = np.arange(T, dtype=np.int16).reshape(T, 1)
    tri = np.triu(np.ones((P, P), np.float32)).astype(ml_dtypes.bfloat16)
    ident = np.eye(P, dtype=np.float32)
    onesb = np.ones((1, P), dtype=ml_dtypes.bfloat16)
    onesf = np.ones((1, P), dtype=np.float32)
    idw_id = np.arange(T, dtype=np.int16).reshape(T // 16, 16).T.copy()
    return dict(iota8=iota8, iotat=iotat, tri=tri, ident=ident,
                onesb=onesb, onesf=onesf, idw_id=idw_id)


def kernel(hidden_states, router_w, router_b, w1, b1, gate, w2, b2,
           sw1, sb1, sgate, sw2, sb2, shared_weight, ln_g, ln_b):
    global _PROGRAM
    if _PROGRAM is None:
        _PROGRAM = build_program()
    nc = _PROGRAM

    bf = ml_dtypes.bfloat16
    x = np.ascontiguousarray(np.asarray(hidden_states, np.float32)
                             .reshape(-1, D))
    shared = dict(
        rw=np.asarray(router_w, np.float32),
        rb=np.asarray(router_b, np.float32).reshape(1, E),
        w1=np.asarray(w1, np.float32).astype(bf),
        w2=np.asarray(w2, np.float32).astype(bf),
        b1=np.asarray(b1, np.float32),
        gate=np.asarray(gate, np.float32),
        b2=np.asarray(b2, np.float32).astype(bf),
        sw1=np.asarray(sw1, np.float32).astype(bf),
        sb1=np.asarray(sb1, np.float32),
        sgate=np.asarray(sgate, np.float32),
        sw2=np.asarray(sw2, np.float32).astype(bf),
        sb2=np.asarray(sb2, np.float32).astype(bf).reshape(1, D),
        shw=np.asarray(shared_weight, np.float32).reshape(1, 1),
        lng=np.asarray(ln_g, np.float32).reshape(1, D),
        lnb=np.asarray(ln_b, np.float32).reshape(1, D),
        **_consts(),
    )
    in_maps = [
        {"x": np.ascontiguousarray(x[c * T:(c + 1) * T]), **shared}
        for c in range(NCORES)
    ]
    res = run_bass_kernel_spmd(nc, in_maps, list(range(NCORES)))
    out = np.concatenate([res.results[c]["out"] for c in range(NCORES)], axis=0)
    return out.reshape(B, S, D).astype(np.float32)


if __name__ == "__main__":
    build_program()
    print("kernel program built OK")


# revision 12
# speedup vs baseline: 1.0447x; 1.0447x over previous
"""Trainium2 Bass kernel for nn_ExpertFFNEnsemble (MoE routing, 8 experts, top-2).

Strategy: data-parallel over tokens (8192 tokens -> 1024/core, 8 cores).
Per core, fully on device:
  router (fp32)  -> top-2 + combine weights (sigmoid of logit gap)
  compaction     -> per-expert capacity buckets via cumsum-ranks + indirect scatter
  dispatch       -> one dma_gather(transpose=True) of all bucket rows (bf16)
  expert FFNs    -> bf16 matmuls, fp32 PSUM, exact-gelu ACT epilogue
  shared expert  -> dense bf16 FFN over the core's 1024 tokens
  combine + LN   -> indirect row gathers of the two bucket outputs + LayerNorm
No cross-core communication; host only shards tokens / casts weights to bf16
and concatenates the per-core output slices.
"""

import sys

sys.path.insert(0, "/opt/trn_rl_repo")

import numpy as np
import ml_dtypes

import concourse.bass as bass
import concourse.mybir as mybir
import concourse.tile as tile
from concourse import bacc
from concourse.bass import IndirectOffsetOnAxis
from concourse.bass_utils import run_bass_kernel_spmd

P = 128
B, S, D, F = 4, 2048, 1024, 4096
F2 = F // 2
E = 8
NCORES = 8
T = (B * S) // NCORES           # 1024 tokens per core
NT = T // P                     # 8 token tiles
ND = D // P                     # 8 d-chunks
NF = F // P                     # 32 f-chunks
NF2 = F2 // P                   # 16 f2-chunks
CAP = 384                       # per-expert token capacity (3 x 128)
NIDX = E * CAP                  # 2560 bucket rows (%128 == 0)
TRASH = NIDX                    # overflow-redirect row
BID_ROWS = NIDX + P             # bucket-id rows incl. trash region
YBK_ROWS = NIDX + P             # fc2 output rows incl. trash region
LN_EPS = 1e-5
FT = mybir.ActivationFunctionType
dt = mybir.dt
AX = mybir.AxisListType
OP = mybir.AluOpType

_PROGRAM = None


def _cap_tiles():
    """Token tiles within one expert bucket: list of (row_offset, rows)."""
    out = []
    r = 0
    while r < CAP:
        n = min(P, CAP - r)
        out.append((r, n))
        r += n
    return out


def build_program():
    nc = bacc.Bacc("TRN2", target_bir_lowering=False, debug=False,
                   num_devices=NCORES)

    def din(name, shape, dtype):
        return nc.dram_tensor(name, list(shape), dtype, kind="ExternalInput").ap()

    x_ap = din("x", [T, D], dt.float32)
    rw_ap = din("rw", [D, E], dt.float32)
    rb_ap = din("rb", [1, E], dt.float32)
    w1_ap = din("w1", [E, F // 512, P, ND, 512], dt.bfloat16)
    w2_ap = din("w2", [E, F, D], dt.bfloat16)
    b1_ap = din("b1", [E, F], dt.float32)
    gate_ap = din("gate", [E, F], dt.float32)
    b2_ap = din("b2", [E, D], dt.bfloat16)
    sw1_ap = din("sw1", [F2 // 512, P, ND, 512], dt.bfloat16)
    sb1_ap = din("sb1", [F2], dt.float32)
    sgate_ap = din("sgate", [F2], dt.float32)
    sw2_ap = din("sw2", [F2, D], dt.bfloat16)
    sb2_ap = din("sb2", [1, D], dt.bfloat16)
    shw_ap = din("shw", [1, 1], dt.float32)
    lng_ap = din("lng", [1, D], dt.float32)
    lnb_ap = din("lnb", [1, D], dt.float32)
    # host-provided constants
    iota8_ap = din("iota8", [P, 8], dt.float32)
    iotat_ap = din("iotat", [T, 1], dt.int16)
    tri_ap = din("tri", [P, P], dt.bfloat16)
    ident_ap = din("ident", [P, P], dt.float32)
    onesb_ap = din("onesb", [1, P], dt.bfloat16)
    onesf_ap = din("onesf", [1, P], dt.float32)
    idw_id_ap = din("idw_id", [16, T // 16], dt.int16)  # wrapped identity idx

    out_ap = nc.dram_tensor("out", [T, D], dt.float32, kind="ExternalOutput").ap()

    xbf_dram = nc.dram_tensor("xbf_i", [T, D], dt.bfloat16).ap()
    bid_dram = nc.dram_tensor("bid_i", [BID_ROWS, 1], dt.int16).ap()
    ybk_dram = nc.dram_tensor("ybk_i", [YBK_ROWS, D], dt.float32).ap()

    with tile.TileContext(nc) as tc:
        with (
            tc.tile_pool(name="persist", bufs=1) as pp,
            tc.tile_pool(name="small", bufs=1) as sp,
        ):
            # ---- constants ----
            iota8 = pp.tile([P, 8], dt.float32)
            nc.sync.dma_start(iota8[:], iota8_ap[:])
            tri = pp.tile([P, P], dt.bfloat16)
            nc.sync.dma_start(tri[:], tri_ap[:])
            ident = pp.tile([P, P], dt.float32)
            nc.sync.dma_start(ident[:], ident_ap[:])
            onesb = pp.tile([1, P], dt.bfloat16)
            nc.sync.dma_start(onesb[:], onesb_ap[:])
            onesf = pp.tile([1, P], dt.float32)
            nc.sync.dma_start(onesf[:], onesf_ap[:])
            rw_sb = pp.tile([P, ND, E], dt.float32)
            nc.sync.dma_start(rw_sb[:], rw_ap.rearrange("(k p) e -> p k e", p=P))
            rb_sb = pp.tile([1, E], dt.float32)
            nc.sync.dma_start(rb_sb[:], rb_ap[:, :])
            eps_t = pp.tile([P, 1], dt.float32)
            nc.vector.memset(eps_t[:], LN_EPS)

            # gate / b1*gate per expert: [128, E, NF]
            gate_sb = pp.tile([P, E, NF], dt.float32)
            nc.sync.dma_start(gate_sb[:], gate_ap.rearrange("e (m p) -> p e m", p=P))
            b1_sb = pp.tile([P, E, NF], dt.float32)
            nc.sync.dma_start(b1_sb[:], b1_ap.rearrange("e (m p) -> p e m", p=P))
            b1g_sb = pp.tile([P, E, NF], dt.float32)
            nc.vector.tensor_mul(b1g_sb[:], b1_sb[:], gate_sb[:])
            b2_sb = pp.tile([1, E, D], dt.bfloat16)
            nc.sync.dma_start(b2_sb[:], b2_ap.rearrange("e d2 -> e d2")[None, :, :])
            sg_sb = pp.tile([P, NF2], dt.float32)
            nc.sync.dma_start(sg_sb[:], sgate_ap.rearrange("(m p) -> p m", p=P))
            sb1_sb = pp.tile([P, NF2], dt.float32)
            nc.sync.dma_start(sb1_sb[:], sb1_ap.rearrange("(m p) -> p m", p=P))
            sb1g_sb = pp.tile([P, NF2], dt.float32)
            nc.vector.tensor_mul(sb1g_sb[:], sb1_sb[:], sg_sb[:])
            sb2_sb = pp.tile([1, D], dt.bfloat16)
            nc.sync.dma_start(sb2_sb[:], sb2_ap[:, :])

            # zero tiles for pre-clearing internal DRAM
            zid = sp.tile([P, BID_ROWS // P], dt.int16, tag="zid")
            nc.vector.memset(zid[:], 0)
            nc.sync.dma_start(
                bid_dram.rearrange("(p c) one -> p (c one)", p=P), zid[:])
            ztrash = sp.tile([P, D], dt.float32, tag="ztrash")
            nc.vector.memset(ztrash[:], 0.0)
            nc.sync.dma_start(ybk_dram[TRASH:TRASH + P, :], ztrash[:])

            shw_sb = sp.tile([1, 1], dt.float32, tag="shw")
            nc.sync.dma_start(shw_sb[:], shw_ap[:, :])
            sig1 = sp.tile([1, 1], dt.float32, tag="sig1")
            nc.scalar.activation(sig1[:], shw_sb[:], FT.Sigmoid)
            sig_bc = pp.tile([P, 1], dt.float32)

            # per-token routing results, kept for the combine phase
            cw_tiles = []
            pos_tiles = []

            # xT bf16, chunk-major: [128, NT, ND, P]
            xTb = pp.tile([P, NT, ND, P], dt.bfloat16)
            # shared-expert output (scaled), kept until combine
            ys = pp.tile([P, NT, D], dt.float32)

            with (
                tc.tile_pool(name="ph0", bufs=2) as p0,
                tc.tile_pool(name="ph0ps", bufs=1, space="PSUM") as p0ps,
            ):
                # -- phase 0 + router, with xTf in its own pool scope --
                with tc.tile_pool(name="xTf_pool", bufs=1) as pxT:
                    psig = p0ps.tile([P, 8], dt.float32, tag="rtr", name="psig",
                                     space="PSUM")
                    nc.tensor.matmul(psig[:, 0:1], lhsT=onesf[:, :],
                                     rhs=sig1[:, :], start=True, stop=True)
                    nc.vector.tensor_copy(sig_bc[:], psig[:, 0:1])

                    xTf = pxT.tile([P, ND, T], dt.float32)
                    for i in range(NT):
                        xt = p0.tile([P, D], dt.float32, tag="xt")
                        nc.sync.dma_start(xt[:], x_ap[i * P:(i + 1) * P, :])
                        xb = p0.tile([P, D], dt.bfloat16, tag="xb")
                        nc.vector.tensor_copy(xb[:], xt[:])
                        nc.sync.dma_start(xbf_dram[i * P:(i + 1) * P, :], xb[:])
                        for k in range(ND):
                            ptr = p0ps.tile([P, P], dt.float32, tag="ptr",
                                            space="PSUM", bufs=2)
                            nc.tensor.transpose(
                                ptr[:], xt[:, k * P:(k + 1) * P], ident[:])
                            nc.vector.tensor_copy(
                                xTf[:, k, i * P:(i + 1) * P], ptr[:])

                    # xTb via identity dma_gather (bf16, transposed)
                    idw_id = p0.tile([P, T // 16], dt.int16, tag="idw_id")
                    for g in range(8):
                        nc.sync.dma_start(idw_id[g * 16:(g + 1) * 16, :],
                                          idw_id_ap[:, :])
                    for j in range(T // P):
                        nc.gpsimd.dma_gather(
                            out_ap=xTb[:, j, :, :],
                            in_ap=xbf_dram[:, :],
                            idxs_ap=idw_id[:, j * 8:(j + 1) * 8],
                            num_idxs=P, num_idxs_reg=P, elem_size=D,
                            transpose=True)

                    # ---- router + compaction ----
                    carry = pp.tile([E, 1], dt.float32)
                    nc.vector.memset(carry[:], 0.0)
                    rank_sb = pp.tile([E, T], dt.float32)

                    for i in range(NT):
                        tsl = slice(i * P, (i + 1) * P)
                        pl = p0ps.tile([P, 8], dt.float32, tag="rtr",
                                       name=f"pl{i}", space="PSUM")
                        for k in range(ND):
                            nc.tensor.matmul(pl[:], lhsT=xTf[:, k, tsl],
                                             rhs=rw_sb[:, k, :],
                                             start=(k == 0), stop=False)
                        nc.tensor.matmul(pl[:], lhsT=onesf[:, :], rhs=rb_sb[:, :],
                                         start=False, stop=True)
                        vals = p0.tile([P, 8], dt.float32, tag="vals")
                        idx = p0.tile([P, 8], dt.uint32, tag="idx")
                        lt = p0.tile([P, 8], dt.float32, tag="lt")
                        nc.vector.tensor_copy(lt[:], pl[:])
                        nc.vector.max_with_indices(vals[:], idx[:], lt[:])

                        d01 = p0.tile([P, 1], dt.float32, tag="d01")
                        nc.vector.tensor_sub(d01[:], vals[:, 0:1], vals[:, 1:2])
                        cw = pp.tile([P, 2], dt.float32, tag=f"cw{i}")
                        nc.scalar.activation(cw[:, 0:1], d01[:], FT.Sigmoid)
                        nc.scalar.activation(cw[:, 1:2], d01[:], FT.Sigmoid,
                                             scale=-1.0)
                        cw_tiles.append(cw)

                        ef = p0.tile([P, 2], dt.float32, tag="ef")
                        nc.vector.tensor_copy(ef[:], idx[:, 0:2])
                        oh0 = p0.tile([P, 8], dt.float32, tag="oh0")
                        oh1 = p0.tile([P, 8], dt.float32, tag="oh1")
                        nc.vector.tensor_tensor(
                            out=oh0[:], in0=ef[:, 0:1].to_broadcast([P, 8]),
                            in1=iota8[:], op=OP.is_equal)
                        nc.vector.tensor_tensor(
                            out=oh1[:], in0=ef[:, 1:2].to_broadcast([P, 8]),
                            in1=iota8[:], op=OP.is_equal)
                        A = p0.tile([P, 8], dt.bfloat16, tag="A")
                        nc.vector.tensor_add(A[:], oh0[:], oh1[:])

                        pr = p0ps.tile([E, P], dt.float32, tag="rtr",
                                       name=f"pr{i}", space="PSUM")
                        nc.tensor.matmul(pr[:], lhsT=A[:], rhs=tri[:],
                                         start=True, stop=True)
                        nc.vector.tensor_scalar_add(rank_sb[:, tsl], pr[:],
                                                    carry[:, 0:1])
                        nc.vector.tensor_copy(
                            carry[:], rank_sb[:, i * P + P - 1:i * P + P])

                        prt = p0ps.tile([P, E], dt.float32, tag="rtr",
                                        name=f"prt{i}", space="PSUM")
                        nc.tensor.transpose(prt[:], rank_sb[:, tsl],
                                            ident[:E, :E])
                        rank_t = p0.tile([P, E], dt.float32, tag="rank_t")
                        nc.vector.tensor_copy(rank_t[:], prt[:])

                        tmp = p0.tile([P, 8], dt.float32, tag="tmp")
                        r0 = p0.tile([P, 1], dt.float32, tag="r0")
                        r1 = p0.tile([P, 1], dt.float32, tag="r1")
                        nc.vector.tensor_mul(tmp[:], oh0[:], rank_t[:])
                        nc.vector.reduce_sum(r0[:], tmp[:], axis=AX.X)
                        nc.vector.tensor_mul(tmp[:], oh1[:], rank_t[:])
                        nc.vector.reduce_sum(r1[:], tmp[:], axis=AX.X)

                        posf = p0.tile([P, 2], dt.float32, tag="posf")
                        nc.vector.tensor_scalar(
                            out=posf[:, 0:1], in0=ef[:, 0:1],
                            scalar1=float(CAP), scalar2=None, op0=OP.mult)
                        nc.vector.tensor_scalar(
                            out=posf[:, 1:2], in0=ef[:, 1:2],
                            scalar1=float(CAP), scalar2=None, op0=OP.mult)
                        nc.vector.scalar_tensor_tensor(
                            out=posf[:, 0:1], in0=r0[:], scalar=-1.0,
                            in1=posf[:, 0:1], op0=OP.add, op1=OP.add)
                        nc.vector.scalar_tensor_tensor(
                            out=posf[:, 1:2], in0=r1[:], scalar=-1.0,
                            in1=posf[:, 1:2], op0=OP.add, op1=OP.add)
                        ovf = p0.tile([P, 2], dt.uint8, tag="ovf")
                        nc.vector.tensor_scalar(
                            out=ovf[:, 0:1], in0=r0[:], scalar1=float(CAP),
                            scalar2=None, op0=OP.is_gt)
                        nc.vector.tensor_scalar(
                            out=ovf[:, 1:2], in0=r1[:], scalar1=float(CAP),
                            scalar2=None, op0=OP.is_gt)
                        trash = p0.tile([P, 2], dt.float32, tag="trash")
                        nc.vector.memset(trash[:], float(TRASH))
                        nc.vector.copy_predicated(posf[:], ovf[:], trash[:])
                        pos_i = pp.tile([P, 2], dt.int32, tag=f"pos{i}")
                        nc.vector.tensor_copy(pos_i[:], posf[:])
                        pos_tiles.append(pos_i)

                        tok16 = p0.tile([P, 1], dt.int16, tag="tok16")
                        nc.sync.dma_start(tok16[:], iotat_ap[tsl, :])
                        for s in range(2):
                            nc.gpsimd.indirect_dma_start(
                                out=bid_dram[:, :],
                                out_offset=IndirectOffsetOnAxis(
                                    ap=pos_i[:, s:s + 1], axis=0),
                                in_=tok16[:, :], in_offset=None)

                # -- shared expert (xTf pool closed; reuses its space) --
                with (
                    tc.tile_pool(name="sw1p", bufs=2) as sw1p,
                    tc.tile_pool(name="sw2p", bufs=1) as sw2p,
                    tc.tile_pool(name="hsTp", bufs=1) as hsTp,
                    tc.tile_pool(name="ps3", bufs=2, space="PSUM") as ps3,
                    tc.tile_pool(name="ps4", bufs=2, space="PSUM") as ps4,
                ):
                    hsT = hsTp.tile([P, NF2, T], dt.bfloat16)
                    for m5 in range(F2 // 512):
                        sw1m = sw1p.tile([P, ND, 512], dt.bfloat16, tag="sw1m")
                        nc.sync.dma_start(sw1m[:], sw1_ap[m5])
                        for mm in range(4):
                            m = m5 * 4 + mm
                            for n in range(2):
                                pm = ps3.tile([P, 512], dt.float32, tag="pm3",
                                              space="PSUM")
                                for k in range(ND):
                                    nc.tensor.matmul(
                                        pm[:],
                                        lhsT=sw1m[:, k, mm * P:(mm + 1) * P],
                                        rhs=xTb[:, 4 * n:4 * n + 4, k, :],
                                        start=(k == 0), stop=(k == ND - 1))
                                nc.scalar.activation(
                                    hsT[:, m, n * 512:(n + 1) * 512], pm[:],
                                    FT.Gelu, bias=sb1g_sb[:, m:m + 1],
                                    scale=sg_sb[:, m:m + 1])

                    sw2_sb = sw2p.tile([P, NF2, D], dt.bfloat16)
                    nc.sync.dma_start(
                        sw2_sb[:], sw2_ap.rearrange("(k p) d2 -> p k d2", p=P))
                    for j in range(NT):
                        jsl = slice(j * P, (j + 1) * P)
                        for n in range(2):
                            pyt = ps4.tile([P, 512], dt.float32, tag="py4",
                                           space="PSUM")
                            for k in range(NF2):
                                nc.tensor.matmul(
                                    pyt[:], lhsT=hsT[:, k, jsl],
                                    rhs=sw2_sb[:, k, n * 512:(n + 1) * 512],
                                    start=(k == 0), stop=False)
                            nc.tensor.matmul(
                                pyt[:], lhsT=onesb[:, :],
                                rhs=sb2_sb[:, n * 512:(n + 1) * 512],
                                start=False, stop=True)
                            # ys = sigmoid(shared_weight) * (fc2s + sb2)
                            nc.scalar.activation(
                                ys[:, j, n * 512:(n + 1) * 512], pyt[:],
                                FT.Copy, scale=sig_bc[:, 0:1])

            # ---- dispatch gather + expert FFNs ----
            with (
                tc.tile_pool(name="gx_pool", bufs=1) as pgx,
                tc.tile_pool(name="w1p", bufs=3) as w1p,
                tc.tile_pool(name="w2p", bufs=3) as w2p,
                tc.tile_pool(name="hTp", bufs=1) as hTp,
                tc.tile_pool(name="ps1", bufs=2, space="PSUM") as ps1,
                tc.tile_pool(name="ps2", bufs=1, space="PSUM") as ps2,
            ):
                gxT = pgx.tile([P, NIDX // P, ND, P], dt.bfloat16)
                idxw = pgx.tile([P, NIDX // 16], dt.int16)
                for g in range(8):
                    nc.sync.dma_start(
                        idxw[g * 16:(g + 1) * 16, :],
                        bid_dram[:NIDX, :].rearrange(
                            "(c p) one -> p (c one)", p=16))
                for j in range(NIDX // P):
                    nc.gpsimd.dma_gather(
                        out_ap=gxT[:, j, :, :],
                        in_ap=xbf_dram[:, :],
                        idxs_ap=idxw[:, j * 8:(j + 1) * 8],
                        num_idxs=P, num_idxs_reg=P, elem_size=D,
                        transpose=True)

                for e in range(E):
                    hT = hTp.tile([P, NF, CAP], dt.bfloat16, tag="hT")
                    # fc1 over 512-wide F chunks
                    for m5 in range(F // 512):
                        w1m = w1p.tile([P, ND, 512], dt.bfloat16, tag="w1m")
                        nc.sync.dma_start(w1m[:], w1_ap[e, m5])
                        for mm in range(4):
                            m = m5 * 4 + mm
                            pm = ps1.tile([P, CAP], dt.float32, tag="pm",
                                          space="PSUM")
                            for k in range(ND):
                                nc.tensor.matmul(
                                    pm[:],
                                    lhsT=w1m[:, k, mm * P:(mm + 1) * P],
                                    rhs=gxT[:, 3 * e:3 * e + 3, k, :],
                                    start=(k == 0), stop=(k == ND - 1))
                            nc.scalar.activation(
                                hT[:, m, :], pm[:], FT.Gelu,
                                bias=b1g_sb[:, e, m:m + 1],
                                scale=gate_sb[:, e, m:m + 1])
                    # fc2: 6 psum tiles held across the k loop
                    pys = [
                        ps2.tile([P, 512], dt.float32, tag=f"py{j}_{n}",
                                 name=f"py_e{e}_{j}_{n}", space="PSUM")
                        for j, (ro, rn) in enumerate(_cap_tiles())
                        for n in range(2)
                    ]
                    for k in range(NF):
                        w2k = w2p.tile([P, D], dt.bfloat16, tag="w2k")
                        nc.sync.dma_start(w2k[:], w2_ap[e, k * P:(k + 1) * P, :])
                        pi = 0
                        for (ro, rn) in _cap_tiles():
                            for n in range(2):
                                nc.tensor.matmul(
                                    pys[pi][:rn, :],
                                    lhsT=hT[:, k, ro:ro + rn],
                                    rhs=w2k[:, n * 512:(n + 1) * 512],
                                    start=(k == 0), stop=False)
                                pi += 1
                    pi = 0
                    for (ro, rn) in _cap_tiles():
                        for n in range(2):
                            nc.tensor.matmul(
                                pys[pi][:rn, :], lhsT=onesb[:, :rn],
                                rhs=b2_sb[:, e, n * 512:(n + 1) * 512],
                                start=False, stop=True)
                            yev = w2p.tile([P, 512], dt.float32, tag="yev",
                                           name=f"yev_{e}_{pi}")
                            nc.vector.tensor_copy(yev[:rn, :], pys[pi][:rn, :])
                            nc.sync.dma_start(
                                ybk_dram[e * CAP + ro:e * CAP + ro + rn,
                                         n * 512:(n + 1) * 512],
                                yev[:rn, :])
                            pi += 1

            # ---- combine + LayerNorm ----
            with tc.tile_pool(name="ph5", bufs=3) as p5:
                lng_bc = pp.tile([P, D], dt.float32)
                nc.sync.dma_start(lng_bc[:], lng_ap.to_broadcast([P, D]))
                lnb_bc = pp.tile([P, D], dt.float32)
                nc.sync.dma_start(lnb_bc[:], lnb_ap.to_broadcast([P, D]))
                for i in range(NT):
                    g0 = p5.tile([P, D], dt.float32, tag="g0")
                    g1 = p5.tile([P, D], dt.float32, tag="g1")
                    nc.gpsimd.indirect_dma_start(
                        out=g0[:], out_offset=None, in_=ybk_dram[:, :],
                        in_offset=IndirectOffsetOnAxis(
                            ap=pos_tiles[i][:, 0:1], axis=0))
                    nc.gpsimd.indirect_dma_start(
                        out=g1[:], out_offset=None, in_=ybk_dram[:, :],
                        in_offset=IndirectOffsetOnAxis(
                            ap=pos_tiles[i][:, 1:2], axis=0))
                    comb = p5.tile([P, D], dt.float32, tag="comb")
                    nc.vector.scalar_tensor_tensor(
                        out=comb[:], in0=g0[:], scalar=cw_tiles[i][:, 0:1],
                        in1=ys[:, i, :], op0=OP.mult, op1=OP.add)
                    nc.vector.scalar_tensor_tensor(
                        out=comb[:], in0=g1[:], scalar=cw_tiles[i][:, 1:2],
                        in1=comb[:], op0=OP.mult, op1=OP.add)
                    mu = p5.tile([P, 1], dt.float32, tag="mu")
                    nc.vector.reduce_sum(mu[:], comb[:], axis=AX.X)
                    nc.vector.tensor_scalar_mul(mu[:], mu[:], 1.0 / D)
                    yc = p5.tile([P, D], dt.float32, tag="yc")
                    nc.vector.tensor_scalar(out=yc[:], in0=comb[:],
                                            scalar1=mu[:, 0:1], scalar2=None,
                                            op0=OP.subtract)
                    sq = p5.tile([P, D], dt.float32, tag="sq")
                    varsum = p5.tile([P, 1], dt.float32, tag="varsum")
                    nc.scalar.activation(sq[:], yc[:], FT.Square,
                                         accum_out=varsum[:])
                    sd = p5.tile([P, 1], dt.float32, tag="sd")
                    nc.scalar.activation(sd[:], varsum[:], FT.Sqrt,
                                         scale=1.0 / D, bias=eps_t[:, 0:1])
                    rinv = p5.tile([P, 1], dt.float32, tag="rinv")
                    nc.vector.reciprocal(rinv[:], sd[:])
                    o1 = p5.tile([P, D], dt.float32, tag="o1")
                    nc.vector.scalar_tensor_tensor(
                        out=o1[:], in0=yc[:], scalar=rinv[:, 0:1],
                        in1=lng_bc[:], op0=OP.mult, op1=OP.mult)
                    nc.vector.tensor_add(o1[:], o1[:], lnb_bc[:])
                    nc.sync.dma_start(out_ap[i * P:(i + 1) * P, :], o1[:])

    nc.compile()
    return nc


def _consts():
    iota8 = np.tile(np.arange(8, dtype=np.float32), (P, 1))
    iotat = np.arange(T, dtype=np.int16).reshape(T, 1)
    tri = np.triu(np.ones((P, P), np.float32)).astype(ml_dtypes.bfloat16)
    ident = np.eye(P, dtype=np.float32)
    onesb = np.ones((1, P), dtype=ml_dtypes.bfloat16)
    onesf = np.ones((1, P), dtype=np.float32)
    idw_id = np.arange(T, dtype=np.int16).reshape(T // 16, 16).T.copy()
    return dict(iota8=iota8, iotat=iotat, tri=tri, ident=ident,
                onesb=onesb, onesf=onesf, idw_id=idw_id)



def _pack_w1(w1f):
    """[E, D, F] f32 -> [E, F//512, P, ND, 512] bf16 (fc1 SBUF tile layout)."""
    bf = ml_dtypes.bfloat16
    return np.ascontiguousarray(
        np.asarray(w1f, np.float32).astype(bf)
        .reshape(E, ND, P, F // 512, 512).transpose(0, 3, 2, 1, 4))


def _pack_sw1(sw1f):
    """[D, F2] f32 -> [F2//512, P, ND, 512] bf16."""
    bf = ml_dtypes.bfloat16
    return np.ascontiguousarray(
        np.asarray(sw1f, np.float32).astype(bf)
        .reshape(ND, P, F2 // 512, 512).transpose(2, 1, 0, 3))


def make_in_maps(inputs):
    """Build the 8 per-core input maps from the full problem inputs."""
    bf = ml_dtypes.bfloat16
    x = np.ascontiguousarray(
        np.asarray(inputs["hidden_states"], np.float32).reshape(-1, D))
    shared = dict(
        rw=np.asarray(inputs["router_w"], np.float32),
        rb=np.asarray(inputs["router_b"], np.float32).reshape(1, E),
        w1=_pack_w1(inputs["w1"]),
        w2=np.asarray(inputs["w2"], np.float32).astype(bf),
        b1=np.asarray(inputs["b1"], np.float32),
        gate=np.asarray(inputs["gate"], np.float32),
        b2=np.asarray(inputs["b2"], np.float32).astype(bf),
        sw1=_pack_sw1(inputs["sw1"]),
        sb1=np.asarray(inputs["sb1"], np.float32),
        sgate=np.asarray(inputs["sgate"], np.float32),
        sw2=np.asarray(inputs["sw2"], np.float32).astype(bf),
        sb2=np.asarray(inputs["sb2"], np.float32).astype(bf).reshape(1, D),
        shw=np.asarray(inputs["shared_weight"], np.float32).reshape(1, 1),
        lng=np.asarray(inputs["ln_g"], np.float32).reshape(1, D),
        lnb=np.asarray(inputs["ln_b"], np.float32).reshape(1, D),
        **_consts(),
    )
    return [{"x": np.ascontiguousarray(x[c * T:(c + 1) * T]), **shared}
            for c in range(NCORES)]


def kernel(hidden_states, router_w, router_b, w1, b1, gate, w2, b2,
           sw1, sb1, sgate, sw2, sb2, shared_weight, ln_g, ln_b):
    global _PROGRAM
    if _PROGRAM is None:
        _PROGRAM = build_program()
    nc = _PROGRAM

    in_maps = make_in_maps(dict(
        hidden_states=hidden_states, router_w=router_w, router_b=router_b,
        w1=w1, b1=b1, gate=gate, w2=w2, b2=b2, sw1=sw1, sb1=sb1, sgate=sgate,
        sw2=sw2, sb2=sb2, shared_weight=shared_weight, ln_g=ln_g, ln_b=ln_b))
    res = run_bass_kernel_spmd(nc, in_maps, list(range(NCORES)))
    out = np.concatenate([res.results[c]["out"] for c in range(NCORES)], axis=0)
    return out.reshape(B, S, D).astype(np.float32)


if __name__ == "__main__":
    build_program()
    print("kernel program built OK")


# revision 17
# speedup vs baseline: 1.0923x; 1.0455x over previous
"""Trainium2 Bass kernel for nn_ExpertFFNEnsemble (MoE routing, 8 experts, top-2).

Strategy: data-parallel over tokens (8192 tokens -> 1024/core, 8 cores).
Per core, fully on device:
  router (fp32)  -> top-2 + combine weights (sigmoid of logit gap)
  compaction     -> per-expert capacity buckets via cumsum-ranks + indirect scatter
  dispatch       -> one dma_gather(transpose=True) of all bucket rows (bf16)
  expert FFNs    -> bf16 matmuls, fp32 PSUM, exact-gelu ACT epilogue
  shared expert  -> dense bf16 FFN over the core's 1024 tokens
  combine + LN   -> indirect row gathers of the two bucket outputs + LayerNorm
No cross-core communication; host only shards tokens / casts weights to bf16
and concatenates the per-core output slices.
"""

import sys

sys.path.insert(0, "/opt/trn_rl_repo")

import numpy as np
import ml_dtypes

import concourse.bass as bass
import concourse.mybir as mybir
import concourse.tile as tile
from concourse import bacc
from concourse.bass import IndirectOffsetOnAxis
from concourse.bass_utils import run_bass_kernel_spmd

P = 128
B, S, D, F = 4, 2048, 1024, 4096
F2 = F // 2
E = 8
NCORES = 8
T = (B * S) // NCORES           # 1024 tokens per core
NT = T // P                     # 8 token tiles
ND = D // P                     # 8 d-chunks
NF = F // P                     # 32 f-chunks
NF2 = F2 // P                   # 16 f2-chunks
CAP = 384                       # per-expert token capacity (3 x 128)
NIDX = E * CAP                  # 2560 bucket rows (%128 == 0)
TRASH = NIDX                    # overflow-redirect row
BID_ROWS = NIDX + P             # bucket-id rows incl. trash region
YBK_ROWS = NIDX + P             # fc2 output rows incl. trash region
LN_EPS = 1e-5
FT = mybir.ActivationFunctionType
dt = mybir.dt
AX = mybir.AxisListType
OP = mybir.AluOpType

_PROGRAM = None


def _cap_tiles():
    """Token tiles within one expert bucket: list of (row_offset, rows)."""
    out = []
    r = 0
    while r < CAP:
        n = min(P, CAP - r)
        out.append((r, n))
        r += n
    return out


def build_program():
    nc = bacc.Bacc("TRN2", target_bir_lowering=False, debug=False,
                   num_devices=NCORES)

    def din(name, shape, dtype):
        return nc.dram_tensor(name, list(shape), dtype, kind="ExternalInput").ap()

    x_ap = din("x", [T, D], dt.float32)
    rw_ap = din("rw", [D, E], dt.float32)
    rb_ap = din("rb", [1, E], dt.float32)
    w1_ap = din("w1", [E, F // 512, P, ND, 512], dt.bfloat16)
    w2_ap = din("w2", [E, F, D], dt.bfloat16)
    b1_ap = din("b1", [E, F], dt.float32)
    gate_ap = din("gate", [E, F], dt.float32)
    b2_ap = din("b2", [E, D], dt.bfloat16)
    sw1_ap = din("sw1", [F2 // 512, P, ND, 512], dt.bfloat16)
    sb1_ap = din("sb1", [F2], dt.float32)
    sgate_ap = din("sgate", [F2], dt.float32)
    sw2_ap = din("sw2", [F2, D], dt.bfloat16)
    sb2_ap = din("sb2", [1, D], dt.bfloat16)
    shw_ap = din("shw", [1, 1], dt.float32)
    lng_ap = din("lng", [1, D], dt.float32)
    lnb_ap = din("lnb", [1, D], dt.float32)
    # host-provided constants
    iota8_ap = din("iota8", [P, 8], dt.float32)
    iotat_ap = din("iotat", [T, 1], dt.int16)
    tri_ap = din("tri", [P, P], dt.bfloat16)
    ident_ap = din("ident", [P, P], dt.float32)
    onesb_ap = din("onesb", [1, P], dt.bfloat16)
    onesf_ap = din("onesf", [1, P], dt.float32)
    idw_id_ap = din("idw_id", [16, T // 16], dt.int16)  # wrapped identity idx

    out_ap = nc.dram_tensor("out", [T, D], dt.float32, kind="ExternalOutput").ap()

    xbf_dram = nc.dram_tensor("xbf_i", [T, D], dt.bfloat16).ap()
    bid_dram = nc.dram_tensor("bid_i", [BID_ROWS, 1], dt.int16).ap()
    ybk_dram = nc.dram_tensor("ybk_i", [YBK_ROWS, D], dt.float32).ap()

    with tile.TileContext(nc) as tc:
        with (
            tc.tile_pool(name="persist", bufs=1) as pp,
            tc.tile_pool(name="small", bufs=1) as sp,
        ):
            # ---- constants ----
            iota8 = pp.tile([P, 8], dt.float32)
            nc.sync.dma_start(iota8[:], iota8_ap[:])
            tri = pp.tile([P, P], dt.bfloat16)
            nc.sync.dma_start(tri[:], tri_ap[:])
            ident = pp.tile([P, P], dt.float32)
            nc.sync.dma_start(ident[:], ident_ap[:])
            onesb = pp.tile([1, P], dt.bfloat16)
            nc.sync.dma_start(onesb[:], onesb_ap[:])
            onesf = pp.tile([1, P], dt.float32)
            nc.sync.dma_start(onesf[:], onesf_ap[:])
            rw_sb = pp.tile([P, ND, E], dt.float32)
            nc.sync.dma_start(rw_sb[:], rw_ap.rearrange("(k p) e -> p k e", p=P))
            rb_sb = pp.tile([1, E], dt.float32)
            nc.sync.dma_start(rb_sb[:], rb_ap[:, :])
            eps_t = pp.tile([P, 1], dt.float32)
            nc.vector.memset(eps_t[:], LN_EPS)

            # gate / b1*gate per expert: [128, E, NF]
            gate_sb = pp.tile([P, E, NF], dt.float32)
            nc.sync.dma_start(gate_sb[:], gate_ap.rearrange("e (m p) -> p e m", p=P))
            b1_sb = pp.tile([P, E, NF], dt.float32)
            nc.sync.dma_start(b1_sb[:], b1_ap.rearrange("e (m p) -> p e m", p=P))
            b1g_sb = pp.tile([P, E, NF], dt.float32)
            nc.vector.tensor_mul(b1g_sb[:], b1_sb[:], gate_sb[:])
            b2_sb = pp.tile([1, E, D], dt.bfloat16)
            nc.sync.dma_start(b2_sb[:], b2_ap.rearrange("e d2 -> e d2")[None, :, :])
            sg_sb = pp.tile([P, NF2], dt.float32)
            nc.sync.dma_start(sg_sb[:], sgate_ap.rearrange("(m p) -> p m", p=P))
            sb1_sb = pp.tile([P, NF2], dt.float32)
            nc.sync.dma_start(sb1_sb[:], sb1_ap.rearrange("(m p) -> p m", p=P))
            sb1g_sb = pp.tile([P, NF2], dt.float32)
            nc.vector.tensor_mul(sb1g_sb[:], sb1_sb[:], sg_sb[:])
            sb2_sb = pp.tile([1, D], dt.bfloat16)
            nc.sync.dma_start(sb2_sb[:], sb2_ap[:, :])

            # zero tiles for pre-clearing internal DRAM
            zid = sp.tile([P, BID_ROWS // P], dt.int16, tag="zid")
            nc.vector.memset(zid[:], 0)
            nc.sync.dma_start(
                bid_dram.rearrange("(p c) one -> p (c one)", p=P), zid[:])
            ztrash = sp.tile([P, D // 2], dt.float32, tag="ztrash")
            nc.vector.memset(ztrash[:], 0.0)
            nc.sync.dma_start(ybk_dram[TRASH:TRASH + P, :D // 2], ztrash[:])
            nc.sync.dma_start(ybk_dram[TRASH:TRASH + P, D // 2:], ztrash[:])

            shw_sb = sp.tile([1, 1], dt.float32, tag="shw")
            nc.sync.dma_start(shw_sb[:], shw_ap[:, :])
            sig1 = sp.tile([1, 1], dt.float32, tag="sig1")
            nc.scalar.activation(sig1[:], shw_sb[:], FT.Sigmoid)
            sig_bc = pp.tile([P, 1], dt.float32)

            # per-token routing results, kept for the combine phase
            cw_tiles = []
            pos_tiles = []

            # xT bf16, chunk-major: [128, NT, ND, P]
            xTb = pp.tile([P, NT, ND, P], dt.bfloat16)
            # shared-expert output (scaled), kept until combine
            ys = pp.tile([P, NT, D], dt.float32)

            with (
                tc.tile_pool(name="ph0", bufs=2) as p0,
                tc.tile_pool(name="ph0ps", bufs=1, space="PSUM") as p0ps,
            ):
                # -- phase 0 + router, with xTf in its own pool scope --
                with tc.tile_pool(name="xTf_pool", bufs=1) as pxT:
                    psig = p0ps.tile([P, 8], dt.float32, tag="rtr", name="psig",
                                     space="PSUM")
                    nc.tensor.matmul(psig[:, 0:1], lhsT=onesf[:, :],
                                     rhs=sig1[:, :], start=True, stop=True)
                    nc.vector.tensor_copy(sig_bc[:], psig[:, 0:1])

                    xTf = pxT.tile([P, ND, T], dt.float32)
                    for i in range(NT):
                        xt = p0.tile([P, D], dt.float32, tag="xt")
                        nc.sync.dma_start(xt[:], x_ap[i * P:(i + 1) * P, :])
                        xb = p0.tile([P, D], dt.bfloat16, tag="xb")
                        nc.vector.tensor_copy(xb[:], xt[:])
                        nc.sync.dma_start(xbf_dram[i * P:(i + 1) * P, :], xb[:])
                        for k in range(ND):
                            ptr = p0ps.tile([P, P], dt.float32, tag="ptr",
                                            space="PSUM", bufs=2)
                            nc.tensor.transpose(
                                ptr[:], xt[:, k * P:(k + 1) * P], ident[:])
                            nc.vector.tensor_copy(
                                xTf[:, k, i * P:(i + 1) * P], ptr[:])

                    # xTb via identity dma_gather (bf16, transposed)
                    idw_id = p0.tile([P, T // 16], dt.int16, tag="idw_id")
                    for g in range(8):
                        nc.sync.dma_start(idw_id[g * 16:(g + 1) * 16, :],
                                          idw_id_ap[:, :])
                    for j in range(T // P):
                        nc.gpsimd.dma_gather(
                            out_ap=xTb[:, j, :, :],
                            in_ap=xbf_dram[:, :],
                            idxs_ap=idw_id[:, j * 8:(j + 1) * 8],
                            num_idxs=P, num_idxs_reg=P, elem_size=D,
                            transpose=True)

                    # ---- router + compaction ----
                    carry = pp.tile([E, 1], dt.float32)
                    nc.vector.memset(carry[:], 0.0)
                    rank_sb = pp.tile([E, T], dt.float32)

                    for i in range(NT):
                        tsl = slice(i * P, (i + 1) * P)
                        pl = p0ps.tile([P, 8], dt.float32, tag="rtr",
                                       name=f"pl{i}", space="PSUM")
                        for k in range(ND):
                            nc.tensor.matmul(pl[:], lhsT=xTf[:, k, tsl],
                                             rhs=rw_sb[:, k, :],
                                             start=(k == 0), stop=False)
                        nc.tensor.matmul(pl[:], lhsT=onesf[:, :], rhs=rb_sb[:, :],
                                         start=False, stop=True)
                        vals = p0.tile([P, 8], dt.float32, tag="vals")
                        idx = p0.tile([P, 8], dt.uint32, tag="idx")
                        lt = p0.tile([P, 8], dt.float32, tag="lt")
                        nc.vector.tensor_copy(lt[:], pl[:])
                        nc.vector.max_with_indices(vals[:], idx[:], lt[:])

                        d01 = p0.tile([P, 1], dt.float32, tag="d01")
                        nc.vector.tensor_sub(d01[:], vals[:, 0:1], vals[:, 1:2])
                        cw = pp.tile([P, 2], dt.float32, tag=f"cw{i}")
                        nc.scalar.activation(cw[:, 0:1], d01[:], FT.Sigmoid)
                        nc.scalar.activation(cw[:, 1:2], d01[:], FT.Sigmoid,
                                             scale=-1.0)
                        cw_tiles.append(cw)

                        ef = p0.tile([P, 2], dt.float32, tag="ef")
                        nc.vector.tensor_copy(ef[:], idx[:, 0:2])
                        oh0 = p0.tile([P, 8], dt.float32, tag="oh0")
                        oh1 = p0.tile([P, 8], dt.float32, tag="oh1")
                        nc.vector.tensor_tensor(
                            out=oh0[:], in0=ef[:, 0:1].to_broadcast([P, 8]),
                            in1=iota8[:], op=OP.is_equal)
                        nc.vector.tensor_tensor(
                            out=oh1[:], in0=ef[:, 1:2].to_broadcast([P, 8]),
                            in1=iota8[:], op=OP.is_equal)
                        A = p0.tile([P, 8], dt.bfloat16, tag="A")
                        nc.vector.tensor_add(A[:], oh0[:], oh1[:])

                        pr = p0ps.tile([E, P], dt.float32, tag="rtr",
                                       name=f"pr{i}", space="PSUM")
                        nc.tensor.matmul(pr[:], lhsT=A[:], rhs=tri[:],
                                         start=True, stop=True)
                        nc.vector.tensor_scalar_add(rank_sb[:, tsl], pr[:],
                                                    carry[:, 0:1])
                        nc.vector.tensor_copy(
                            carry[:], rank_sb[:, i * P + P - 1:i * P + P])

                        prt = p0ps.tile([P, E], dt.float32, tag="rtr",
                                        name=f"prt{i}", space="PSUM")
                        nc.tensor.transpose(prt[:], rank_sb[:, tsl],
                                            ident[:E, :E])
                        rank_t = p0.tile([P, E], dt.float32, tag="rank_t")
                        nc.vector.tensor_copy(rank_t[:], prt[:])

                        tmp = p0.tile([P, 8], dt.float32, tag="tmp")
                        r0 = p0.tile([P, 1], dt.float32, tag="r0")
                        r1 = p0.tile([P, 1], dt.float32, tag="r1")
                        nc.vector.tensor_mul(tmp[:], oh0[:], rank_t[:])
                        nc.vector.reduce_sum(r0[:], tmp[:], axis=AX.X)
                        nc.vector.tensor_mul(tmp[:], oh1[:], rank_t[:])
                        nc.vector.reduce_sum(r1[:], tmp[:], axis=AX.X)

                        posf = p0.tile([P, 2], dt.float32, tag="posf")
                        nc.vector.tensor_scalar(
                            out=posf[:, 0:1], in0=ef[:, 0:1],
                            scalar1=float(CAP), scalar2=None, op0=OP.mult)
                        nc.vector.tensor_scalar(
                            out=posf[:, 1:2], in0=ef[:, 1:2],
                            scalar1=float(CAP), scalar2=None, op0=OP.mult)
                        nc.vector.scalar_tensor_tensor(
                            out=posf[:, 0:1], in0=r0[:], scalar=-1.0,
                            in1=posf[:, 0:1], op0=OP.add, op1=OP.add)
                        nc.vector.scalar_tensor_tensor(
                            out=posf[:, 1:2], in0=r1[:], scalar=-1.0,
                            in1=posf[:, 1:2], op0=OP.add, op1=OP.add)
                        ovf = p0.tile([P, 2], dt.uint8, tag="ovf")
                        nc.vector.tensor_scalar(
                            out=ovf[:, 0:1], in0=r0[:], scalar1=float(CAP),
                            scalar2=None, op0=OP.is_gt)
                        nc.vector.tensor_scalar(
                            out=ovf[:, 1:2], in0=r1[:], scalar1=float(CAP),
                            scalar2=None, op0=OP.is_gt)
                        trash = p0.tile([P, 2], dt.float32, tag="trash")
                        nc.vector.memset(trash[:], float(TRASH))
                        nc.vector.copy_predicated(posf[:], ovf[:], trash[:])
                        pos_i = pp.tile([P, 2], dt.int32, tag=f"pos{i}")
                        nc.vector.tensor_copy(pos_i[:], posf[:])
                        pos_tiles.append(pos_i)

                        tok16 = p0.tile([P, 1], dt.int16, tag="tok16")
                        nc.sync.dma_start(tok16[:], iotat_ap[tsl, :])
                        for s in range(2):
                            nc.gpsimd.indirect_dma_start(
                                out=bid_dram[:, :],
                                out_offset=IndirectOffsetOnAxis(
                                    ap=pos_i[:, s:s + 1], axis=0),
                                in_=tok16[:, :], in_offset=None)

            # ---- dispatch gather (gpsimd/DMA) overlaps shared expert (PE) ----
            with tc.tile_pool(name="gx_pool", bufs=1) as pgx:
                gxT = pgx.tile([P, NIDX // P, ND, P], dt.bfloat16)
                idxw = pgx.tile([P, NIDX // 16], dt.int16)
                for g in range(8):
                    nc.sync.dma_start(
                        idxw[g * 16:(g + 1) * 16, :],
                        bid_dram[:NIDX, :].rearrange(
                            "(c p) one -> p (c one)", p=16))
                for j in range(NIDX // P):
                    nc.gpsimd.dma_gather(
                        out_ap=gxT[:, j, :, :],
                        in_ap=xbf_dram[:, :],
                        idxs_ap=idxw[:, j * 8:(j + 1) * 8],
                        num_idxs=P, num_idxs_reg=P, elem_size=D,
                        transpose=True)

                # -- shared expert --
                with (
                    tc.tile_pool(name="sw1p", bufs=1) as sw1p,
                    tc.tile_pool(name="sw2p", bufs=1) as sw2p,
                    tc.tile_pool(name="hsTp", bufs=1) as hsTp,
                    tc.tile_pool(name="ps3", bufs=2, space="PSUM") as ps3,
                    tc.tile_pool(name="ps4", bufs=2, space="PSUM") as ps4,
                ):
                    hsT = hsTp.tile([P, NF2, T], dt.bfloat16)
                    for m5 in range(F2 // 512):
                        sw1m = sw1p.tile([P, ND, 512], dt.bfloat16, tag="sw1m")
                        nc.scalar.dma_start(sw1m[:], sw1_ap[m5])
                        for mm in range(4):
                            m = m5 * 4 + mm
                            for n in range(2):
                                pm = ps3.tile([P, 512], dt.float32, tag="pm3",
                                              space="PSUM")
                                for k in range(ND):
                                    nc.tensor.matmul(
                                        pm[:],
                                        lhsT=sw1m[:, k, mm * P:(mm + 1) * P],
                                        rhs=xTb[:, 4 * n:4 * n + 4, k, :],
                                        start=(k == 0), stop=(k == ND - 1))
                                nc.scalar.activation(
                                    hsT[:, m, n * 512:(n + 1) * 512], pm[:],
                                    FT.Gelu, bias=sb1g_sb[:, m:m + 1],
                                    scale=sg_sb[:, m:m + 1])

                    sw2_sb = sw2p.tile([P, NF2, D], dt.bfloat16)
                    nc.scalar.dma_start(
                        sw2_sb[:], sw2_ap.rearrange("(k p) d2 -> p k d2", p=P))
                    for j in range(NT):
                        jsl = slice(j * P, (j + 1) * P)
                        for n in range(2):
                            pyt = ps4.tile([P, 512], dt.float32, tag="py4",
                                           space="PSUM")
                            for k in range(NF2):
                                nc.tensor.matmul(
                                    pyt[:], lhsT=hsT[:, k, jsl],
                                    rhs=sw2_sb[:, k, n * 512:(n + 1) * 512],
                                    start=(k == 0), stop=False)
                            nc.tensor.matmul(
                                pyt[:], lhsT=onesb[:, :],
                                rhs=sb2_sb[:, n * 512:(n + 1) * 512],
                                start=False, stop=True)
                            # ys = sigmoid(shared_weight) * (fc2s + sb2)
                            nc.scalar.activation(
                                ys[:, j, n * 512:(n + 1) * 512], pyt[:],
                                FT.Copy, scale=sig_bc[:, 0:1])

                # ---- expert FFNs ----
                with (
                    tc.tile_pool(name="w1p", bufs=3) as w1p,
                    tc.tile_pool(name="w2p", bufs=3) as w2p,
                    tc.tile_pool(name="hTp", bufs=1) as hTp,
                    tc.tile_pool(name="ps1", bufs=2, space="PSUM") as ps1,
                    tc.tile_pool(name="ps2", bufs=1, space="PSUM") as ps2,
                ):
                  for e in range(E):
                    hT = hTp.tile([P, NF, CAP], dt.bfloat16, tag="hT")
                    # fc1 over 512-wide F chunks
                    for m5 in range(F // 512):
                        w1m = w1p.tile([P, ND, 512], dt.bfloat16, tag="w1m")
                        nc.scalar.dma_start(w1m[:], w1_ap[e, m5])
                        for mm in range(4):
                            m = m5 * 4 + mm
                            pm = ps1.tile([P, CAP], dt.float32, tag="pm",
                                          space="PSUM")
                            for k in range(ND):
                                nc.tensor.matmul(
                                    pm[:],
                                    lhsT=w1m[:, k, mm * P:(mm + 1) * P],
                                    rhs=gxT[:, 3 * e:3 * e + 3, k, :],
                                    start=(k == 0), stop=(k == ND - 1))
                            nc.scalar.activation(
                                hT[:, m, :], pm[:], FT.Gelu,
                                bias=b1g_sb[:, e, m:m + 1],
                                scale=gate_sb[:, e, m:m + 1])
                    # fc2: 6 psum tiles held across the k loop
                    pys = [
                        ps2.tile([P, 512], dt.float32, tag=f"py{j}_{n}",
                                 name=f"py_e{e}_{j}_{n}", space="PSUM")
                        for j, (ro, rn) in enumerate(_cap_tiles())
                        for n in range(2)
                    ]
                    for k in range(NF):
                        w2k = w2p.tile([P, D], dt.bfloat16, tag="w2k")
                        nc.scalar.dma_start(w2k[:], w2_ap[e, k * P:(k + 1) * P, :])
                        pi = 0
                        for (ro, rn) in _cap_tiles():
                            for n in range(2):
                                nc.tensor.matmul(
                                    pys[pi][:rn, :],
                                    lhsT=hT[:, k, ro:ro + rn],
                                    rhs=w2k[:, n * 512:(n + 1) * 512],
                                    start=(k == 0), stop=False)
                                pi += 1
                    pi = 0
                    for (ro, rn) in _cap_tiles():
                        for n in range(2):
                            nc.tensor.matmul(
                                pys[pi][:rn, :], lhsT=onesb[:, :rn],
                                rhs=b2_sb[:, e, n * 512:(n + 1) * 512],
                                start=False, stop=True)
                            yev = w2p.tile([P, 512], dt.float32, tag="yev",
                                           name=f"yev_{e}_{pi}")
                            nc.vector.tensor_copy(yev[:rn, :], pys[pi][:rn, :])
                            nc.sync.dma_start(
                                ybk_dram[e * CAP + ro:e * CAP + ro + rn,
                                         n * 512:(n + 1) * 512],
                                yev[:rn, :])
                            pi += 1

            # ---- combine + LayerNorm ----
            with tc.tile_pool(name="ph5", bufs=3) as p5:
                lng_bc = pp.tile([P, D], dt.float32)
                nc.sync.dma_start(lng_bc[:], lng_ap.to_broadcast([P, D]))
                lnb_bc = pp.tile([P, D], dt.float32)
                nc.sync.dma_start(lnb_bc[:], lnb_ap.to_broadcast([P, D]))
                for i in range(NT):
                    g0 = p5.tile([P, D], dt.float32, tag="g0")
                    g1 = p5.tile([P, D], dt.float32, tag="g1")
                    nc.gpsimd.indirect_dma_start(
                        out=g0[:], out_offset=None, in_=ybk_dram[:, :],
                        in_offset=IndirectOffsetOnAxis(
                            ap=pos_tiles[i][:, 0:1], axis=0))
                    nc.gpsimd.indirect_dma_start(
                        out=g1[:], out_offset=None, in_=ybk_dram[:, :],
                        in_offset=IndirectOffsetOnAxis(
                            ap=pos_tiles[i][:, 1:2], axis=0))
                    comb = p5.tile([P, D], dt.float32, tag="comb")
                    nc.vector.scalar_tensor_tensor(
                        out=comb[:], in0=g0[:], scalar=cw_tiles[i][:, 0:1],
                        in1=ys[:, i, :], op0=OP.mult, op1=OP.add)
                    nc.vector.scalar_tensor_tensor(
                        out=comb[:], in0=g1[:], scalar=cw_tiles[i][:, 1:2],
                        in1=comb[:], op0=OP.mult, op1=OP.add)
                    mu = p5.tile([P, 1], dt.float32, tag="mu")
                    nc.vector.reduce_sum(mu[:], comb[:], axis=AX.X)
                    nmu = p5.tile([P, 1], dt.float32, tag="nmu")
                    nc.vector.tensor_scalar_mul(nmu[:], mu[:], -1.0 / D)
                    yc = p5.tile([P, D], dt.float32, tag="yc")
                    nc.scalar.activation(yc[:], comb[:], FT.Identity,
                                         bias=nmu[:, 0:1])
                    sq = p5.tile([P, D], dt.float32, tag="sq")
                    varsum = p5.tile([P, 1], dt.float32, tag="varsum")
                    nc.scalar.activation(sq[:], yc[:], FT.Square,
                                         accum_out=varsum[:])
                    sd = p5.tile([P, 1], dt.float32, tag="sd")
                    nc.scalar.activation(sd[:], varsum[:], FT.Sqrt,
                                         scale=1.0 / D, bias=eps_t[:, 0:1])
                    rinv = p5.tile([P, 1], dt.float32, tag="rinv")
                    nc.vector.reciprocal(rinv[:], sd[:])
                    o1 = p5.tile([P, D], dt.float32, tag="o1")
                    nc.vector.scalar_tensor_tensor(
                        out=o1[:], in0=yc[:], scalar=rinv[:, 0:1],
                        in1=lng_bc[:], op0=OP.mult, op1=OP.mult)
                    nc.vector.tensor_add(o1[:], o1[:], lnb_bc[:])
                    nc.sync.dma_start(out_ap[i * P:(i + 1) * P, :], o1[:])

    nc.compile()
    return nc


def _consts():
    iota8 = np.tile(np.arange(8, dtype=np.float32), (P, 1))
    iotat = np.arange(T, dtype=np.int16).reshape(T, 1)
    tri = np.triu(np.ones((P, P), np.float32)).astype(ml_dtypes.bfloat16)
    ident = np.eye(P, dtype=np.float32)
    onesb = np.ones((1, P), dtype=ml_dtypes.bfloat16)
    onesf = np.ones((1, P), dtype=np.float32)
    idw_id = np.arange(T, dtype=np.int16).reshape(T // 16, 16).T.copy()
    return dict(iota8=iota8, iotat=iotat, tri=tri, ident=ident,
                onesb=onesb, onesf=onesf, idw_id=idw_id)



def _pack_w1(w1f):
    """[E, D, F] f32 -> [E, F//512, P, ND, 512] bf16 (fc1 SBUF tile layout)."""
    bf = ml_dtypes.bfloat16
    return np.ascontiguousarray(
        np.asarray(w1f, np.float32).astype(bf)
        .reshape(E, ND, P, F // 512, 512).transpose(0, 3, 2, 1, 4))


def _pack_sw1(sw1f):
    """[D, F2] f32 -> [F2//512, P, ND, 512] bf16."""
    bf = ml_dtypes.bfloat16
    return np.ascontiguousarray(
        np.asarray(sw1f, np.float32).astype(bf)
        .reshape(ND, P, F2 // 512, 512).transpose(2, 1, 0, 3))


def make_in_maps(inputs):
    """Build the 8 per-core input maps from the full problem inputs."""
    bf = ml_dtypes.bfloat16
    x = np.ascontiguousarray(
        np.asarray(inputs["hidden_states"], np.float32).reshape(-1, D))
    shared = dict(
        rw=np.asarray(inputs["router_w"], np.float32),
        rb=np.asarray(inputs["router_b"], np.float32).reshape(1, E),
        w1=_pack_w1(inputs["w1"]),
        w2=np.asarray(inputs["w2"], np.float32).astype(bf),
        b1=np.asarray(inputs["b1"], np.float32),
        gate=np.asarray(inputs["gate"], np.float32),
        b2=np.asarray(inputs["b2"], np.float32).astype(bf),
        sw1=_pack_sw1(inputs["sw1"]),
        sb1=np.asarray(inputs["sb1"], np.float32),
        sgate=np.asarray(inputs["sgate"], np.float32),
        sw2=np.asarray(inputs["sw2"], np.float32).astype(bf),
        sb2=np.asarray(inputs["sb2"], np.float32).astype(bf).reshape(1, D),
        shw=np.asarray(inputs["shared_weight"], np.float32).reshape(1, 1),
        lng=np.asarray(inputs["ln_g"], np.float32).reshape(1, D),
        lnb=np.asarray(inputs["ln_b"], np.float32).reshape(1, D),
        **_consts(),
    )
    return [{"x": np.ascontiguousarray(x[c * T:(c + 1) * T]), **shared}
            for c in range(NCORES)]


def kernel(hidden_states, router_w, router_b, w1, b1, gate, w2, b2,
           sw1, sb1, sgate, sw2, sb2, shared_weight, ln_g, ln_b):
    global _PROGRAM
    if _PROGRAM is None:
        _PROGRAM = build_program()
    nc = _PROGRAM

    in_maps = make_in_maps(dict(
        hidden_states=hidden_states, router_w=router_w, router_b=router_b,
        w1=w1, b1=b1, gate=gate, w2=w2, b2=b2, sw1=sw1, sb1=sb1, sgate=sgate,
        sw2=sw2, sb2=sb2, shared_weight=shared_weight, ln_g=ln_g, ln_b=ln_b))
    res = run_bass_kernel_spmd(nc, in_maps, list(range(NCORES)))
    out = np.concatenate([res.results[c]["out"] for c in range(NCORES)], axis=0)
    return out.reshape(B, S, D).astype(np.float32)


if __name__ == "__main__":
    build_program()
    print("kernel program built OK")


# revision 19
# speedup vs baseline: 1.1839x; 1.0838x over previous
"""Trainium2 Bass kernel for nn_ExpertFFNEnsemble (MoE routing, 8 experts, top-2).

Strategy: data-parallel over tokens (8192 tokens -> 1024/core, 8 cores).
Per core, fully on device:
  router (fp32)  -> top-2 + combine weights (sigmoid of logit gap)
  compaction     -> per-expert capacity buckets via cumsum-ranks + indirect scatter
  dispatch       -> one dma_gather(transpose=True) of all bucket rows (bf16)
  expert FFNs    -> bf16 matmuls, fp32 PSUM, exact-gelu ACT epilogue
  shared expert  -> dense bf16 FFN over the core's 1024 tokens
  combine + LN   -> indirect row gathers of the two bucket outputs + LayerNorm
No cross-core communication; host only shards tokens / casts weights to bf16
and concatenates the per-core output slices.
"""

import sys

sys.path.insert(0, "/opt/trn_rl_repo")

import numpy as np
import ml_dtypes

import concourse.bass as bass
import concourse.mybir as mybir
import concourse.tile as tile
from concourse import bacc
from concourse.bass import IndirectOffsetOnAxis
from concourse.bass_utils import run_bass_kernel_spmd

P = 128
B, S, D, F = 4, 2048, 1024, 4096
F2 = F // 2
E = 8
NCORES = 8
T = (B * S) // NCORES           # 1024 tokens per core
NT = T // P                     # 8 token tiles
ND = D // P                     # 8 d-chunks
NF = F // P                     # 32 f-chunks
NF2 = F2 // P                   # 16 f2-chunks
CAP = 320                       # per-expert token capacity (2.5 x 128)
NIDX = E * CAP                  # 2560 bucket rows (%128 == 0)
TRASH = NIDX                    # overflow-redirect row
BID_ROWS = NIDX + P             # bucket-id rows incl. trash region
YBK_ROWS = NIDX + P             # fc2 output rows incl. trash region
LN_EPS = 1e-5
FT = mybir.ActivationFunctionType
dt = mybir.dt
AX = mybir.AxisListType
OP = mybir.AluOpType

_PROGRAM = None


def _fc1_segs(e):
    """Moving-operand segments of expert e's bucket in the chunk-major gxT.

    Returns (kind, chunk, a, pos, take): kind 'full' -> a = #chunks,
    kind 'part' -> a = in-chunk offset.
    """
    segs = []
    pos = 0
    start = e * CAP
    while pos < CAP:
        c, off = divmod(start + pos, P)
        if off == 0 and (CAP - pos) >= P:
            n = (CAP - pos) // P
            segs.append(("full", c, n, pos, n * P))
            pos += n * P
        else:
            take = min(P - off, CAP - pos)
            segs.append(("part", c, off, pos, take))
            pos += take
    return segs


def _cap_tiles():
    """Token tiles within one expert bucket: list of (row_offset, rows)."""
    out = []
    r = 0
    while r < CAP:
        n = min(P, CAP - r)
        out.append((r, n))
        r += n
    return out


def build_program():
    nc = bacc.Bacc("TRN2", target_bir_lowering=False, debug=False,
                   num_devices=NCORES)

    def din(name, shape, dtype):
        return nc.dram_tensor(name, list(shape), dtype, kind="ExternalInput").ap()

    x_ap = din("x", [T, D], dt.float32)
    rw_ap = din("rw", [D, E], dt.float32)
    rb_ap = din("rb", [1, E], dt.float32)
    w1_ap = din("w1", [E, F // 512, P, ND, 512], dt.bfloat16)
    w2_ap = din("w2", [E, F, D], dt.bfloat16)
    b1_ap = din("b1", [P, E, NF], dt.float32)
    gate_ap = din("gate", [P, E, NF], dt.float32)
    b2_ap = din("b2", [E, D], dt.bfloat16)
    sw1_ap = din("sw1", [F2 // 512, P, ND, 512], dt.bfloat16)
    sb1_ap = din("sb1", [P, NF2], dt.float32)
    sgate_ap = din("sgate", [P, NF2], dt.float32)
    sw2_ap = din("sw2", [F2, D], dt.bfloat16)
    sb2_ap = din("sb2", [1, D], dt.bfloat16)
    shw_ap = din("shw", [1, 1], dt.float32)
    lng_ap = din("lng", [1, D], dt.float32)
    lnb_ap = din("lnb", [1, D], dt.float32)
    # host-provided constants
    iota8_ap = din("iota8", [P, 8], dt.float32)
    iotat_ap = din("iotat", [T, 1], dt.int16)
    tri_ap = din("tri", [P, P], dt.bfloat16)
    ident_ap = din("ident", [P, P], dt.float32)
    onesb_ap = din("onesb", [1, P], dt.bfloat16)
    onesf_ap = din("onesf", [1, P], dt.float32)
    idw_id_ap = din("idw_id", [16, T // 16], dt.int16)  # wrapped identity idx

    out_ap = nc.dram_tensor("out", [T, D], dt.float32, kind="ExternalOutput").ap()

    xbf_dram = nc.dram_tensor("xbf_i", [T, D], dt.bfloat16).ap()
    bid_dram = nc.dram_tensor("bid_i", [BID_ROWS, 1], dt.int16).ap()
    ybk_dram = nc.dram_tensor("ybk_i", [YBK_ROWS, D], dt.float32).ap()

    with tile.TileContext(nc) as tc:
        with (
            tc.tile_pool(name="persist", bufs=1) as pp,
            tc.tile_pool(name="small", bufs=1) as sp,
        ):
            # ---- constants ----
            iota8 = pp.tile([P, 8], dt.float32)
            nc.sync.dma_start(iota8[:], iota8_ap[:])
            tri = pp.tile([P, P], dt.bfloat16)
            nc.sync.dma_start(tri[:], tri_ap[:])
            ident = pp.tile([P, P], dt.float32)
            nc.sync.dma_start(ident[:], ident_ap[:])
            onesb = pp.tile([1, P], dt.bfloat16)
            nc.sync.dma_start(onesb[:], onesb_ap[:])
            onesf = pp.tile([1, P], dt.float32)
            nc.sync.dma_start(onesf[:], onesf_ap[:])
            rw_sb = pp.tile([P, ND, E], dt.float32)
            nc.sync.dma_start(rw_sb[:], rw_ap.rearrange("(k p) e -> p k e", p=P))
            rb_sb = pp.tile([1, E], dt.float32)
            nc.sync.dma_start(rb_sb[:], rb_ap[:, :])
            eps_t = pp.tile([P, 1], dt.float32)
            nc.vector.memset(eps_t[:], LN_EPS)

            # gate / b1*gate per expert: [128, E, NF]
            gate_sb = pp.tile([P, E, NF], dt.float32)
            nc.sync.dma_start(gate_sb[:], gate_ap[:])
            b1_sb = pp.tile([P, E, NF], dt.float32)
            nc.sync.dma_start(b1_sb[:], b1_ap[:])
            b1g_sb = pp.tile([P, E, NF], dt.float32)
            nc.vector.tensor_mul(b1g_sb[:], b1_sb[:], gate_sb[:])
            b2_sb = pp.tile([1, E, D], dt.bfloat16)
            nc.sync.dma_start(b2_sb[:], b2_ap.rearrange("e d2 -> e d2")[None, :, :])
            sg_sb = pp.tile([P, NF2], dt.float32)
            nc.sync.dma_start(sg_sb[:], sgate_ap[:])
            sb1_sb = pp.tile([P, NF2], dt.float32)
            nc.sync.dma_start(sb1_sb[:], sb1_ap[:])
            sb1g_sb = pp.tile([P, NF2], dt.float32)
            nc.vector.tensor_mul(sb1g_sb[:], sb1_sb[:], sg_sb[:])
            sb2_sb = pp.tile([1, D], dt.bfloat16)
            nc.sync.dma_start(sb2_sb[:], sb2_ap[:, :])

            # zero tiles for pre-clearing internal DRAM
            zid = sp.tile([P, BID_ROWS // P], dt.int16, tag="zid")
            nc.vector.memset(zid[:], 0)
            nc.sync.dma_start(
                bid_dram.rearrange("(p c) one -> p (c one)", p=P), zid[:])
            ztrash = sp.tile([P, D // 2], dt.float32, tag="ztrash")
            nc.vector.memset(ztrash[:], 0.0)
            nc.sync.dma_start(ybk_dram[TRASH:TRASH + P, :D // 2], ztrash[:])
            nc.sync.dma_start(ybk_dram[TRASH:TRASH + P, D // 2:], ztrash[:])

            shw_sb = sp.tile([1, 1], dt.float32, tag="shw")
            nc.sync.dma_start(shw_sb[:], shw_ap[:, :])
            sig1 = sp.tile([1, 1], dt.float32, tag="sig1")
            nc.scalar.activation(sig1[:], shw_sb[:], FT.Sigmoid)
            sig_bc = pp.tile([P, 1], dt.float32)

            # per-token routing results, kept for the combine phase
            cw_tiles = []
            pos_tiles = []

            # xT bf16, chunk-major: [128, NT, ND, P]
            xTb = pp.tile([P, NT, ND, P], dt.bfloat16)
            # shared-expert output (scaled), kept until combine
            ys = pp.tile([P, NT, D], dt.float32)

            with (
                tc.tile_pool(name="ph0", bufs=2) as p0,
                tc.tile_pool(name="ph0ps", bufs=1, space="PSUM") as p0ps,
            ):
                # -- phase 0 + router, with xTf in its own pool scope --
                with tc.tile_pool(name="xTf_pool", bufs=1) as pxT:
                    psig = p0ps.tile([P, 8], dt.float32, tag="rtr", name="psig",
                                     space="PSUM")
                    nc.tensor.matmul(psig[:, 0:1], lhsT=onesf[:, :],
                                     rhs=sig1[:, :], start=True, stop=True)
                    nc.vector.tensor_copy(sig_bc[:], psig[:, 0:1])

                    xTf = pxT.tile([P, ND, T], dt.float32)
                    for i in range(NT):
                        xt = p0.tile([P, D], dt.float32, tag="xt")
                        nc.sync.dma_start(xt[:], x_ap[i * P:(i + 1) * P, :])
                        xb = p0.tile([P, D], dt.bfloat16, tag="xb")
                        nc.vector.tensor_copy(xb[:], xt[:])
                        nc.sync.dma_start(xbf_dram[i * P:(i + 1) * P, :], xb[:])
                        for k in range(ND):
                            ptr = p0ps.tile([P, P], dt.float32, tag="ptr",
                                            space="PSUM", bufs=2)
                            nc.tensor.transpose(
                                ptr[:], xt[:, k * P:(k + 1) * P], ident[:])
                            nc.vector.tensor_copy(
                                xTf[:, k, i * P:(i + 1) * P], ptr[:])

                    # xTb via identity dma_gather (bf16, transposed)
                    idw_id = p0.tile([P, T // 16], dt.int16, tag="idw_id")
                    for g in range(8):
                        nc.sync.dma_start(idw_id[g * 16:(g + 1) * 16, :],
                                          idw_id_ap[:, :])
                    for j in range(T // P):
                        nc.gpsimd.dma_gather(
                            out_ap=xTb[:, j, :, :],
                            in_ap=xbf_dram[:, :],
                            idxs_ap=idw_id[:, j * 8:(j + 1) * 8],
                            num_idxs=P, num_idxs_reg=P, elem_size=D,
                            transpose=True)

                    # ---- router + compaction ----
                    carry = pp.tile([E, 1], dt.float32)
                    nc.vector.memset(carry[:], 0.0)
                    rank_sb = pp.tile([E, T], dt.float32)

                    for i in range(NT):
                        tsl = slice(i * P, (i + 1) * P)
                        pl = p0ps.tile([P, 8], dt.float32, tag="rtr",
                                       name=f"pl{i}", space="PSUM")
                        for k in range(ND):
                            nc.tensor.matmul(pl[:], lhsT=xTf[:, k, tsl],
                                             rhs=rw_sb[:, k, :],
                                             start=(k == 0), stop=False)
                        nc.tensor.matmul(pl[:], lhsT=onesf[:, :], rhs=rb_sb[:, :],
                                         start=False, stop=True)
                        vals = p0.tile([P, 8], dt.float32, tag="vals")
                        idx = p0.tile([P, 8], dt.uint32, tag="idx")
                        lt = p0.tile([P, 8], dt.float32, tag="lt")
                        nc.vector.tensor_copy(lt[:], pl[:])
                        nc.vector.max_with_indices(vals[:], idx[:], lt[:])

                        d01 = p0.tile([P, 1], dt.float32, tag="d01")
                        nc.vector.tensor_sub(d01[:], vals[:, 0:1], vals[:, 1:2])
                        cw = pp.tile([P, 2], dt.float32, tag=f"cw{i}")
                        nc.scalar.activation(cw[:, 0:1], d01[:], FT.Sigmoid)
                        nc.scalar.activation(cw[:, 1:2], d01[:], FT.Sigmoid,
                                             scale=-1.0)
                        cw_tiles.append(cw)

                        ef = p0.tile([P, 2], dt.float32, tag="ef")
                        nc.vector.tensor_copy(ef[:], idx[:, 0:2])
                        oh0 = p0.tile([P, 8], dt.float32, tag="oh0")
                        oh1 = p0.tile([P, 8], dt.float32, tag="oh1")
                        nc.vector.tensor_tensor(
                            out=oh0[:], in0=ef[:, 0:1].to_broadcast([P, 8]),
                            in1=iota8[:], op=OP.is_equal)
                        nc.vector.tensor_tensor(
                            out=oh1[:], in0=ef[:, 1:2].to_broadcast([P, 8]),
                            in1=iota8[:], op=OP.is_equal)
                        A = p0.tile([P, 8], dt.bfloat16, tag="A")
                        nc.vector.tensor_add(A[:], oh0[:], oh1[:])

                        pr = p0ps.tile([E, P], dt.float32, tag="rtr",
                                       name=f"pr{i}", space="PSUM")
                        nc.tensor.matmul(pr[:], lhsT=A[:], rhs=tri[:],
                                         start=True, stop=True)
                        nc.vector.tensor_scalar_add(rank_sb[:, tsl], pr[:],
                                                    carry[:, 0:1])
                        nc.vector.tensor_copy(
                            carry[:], rank_sb[:, i * P + P - 1:i * P + P])

                        prt = p0ps.tile([P, E], dt.float32, tag="rtr",
                                        name=f"prt{i}", space="PSUM")
                        nc.tensor.transpose(prt[:], rank_sb[:, tsl],
                                            ident[:E, :E])
                        rank_t = p0.tile([P, E], dt.float32, tag="rank_t")
                        nc.vector.tensor_copy(rank_t[:], prt[:])

                        tmp = p0.tile([P, 8], dt.float32, tag="tmp")
                        r0 = p0.tile([P, 1], dt.float32, tag="r0")
                        r1 = p0.tile([P, 1], dt.float32, tag="r1")
                        nc.vector.tensor_mul(tmp[:], oh0[:], rank_t[:])
                        nc.vector.reduce_sum(r0[:], tmp[:], axis=AX.X)
                        nc.vector.tensor_mul(tmp[:], oh1[:], rank_t[:])
                        nc.vector.reduce_sum(r1[:], tmp[:], axis=AX.X)

                        posf = p0.tile([P, 2], dt.float32, tag="posf")
                        nc.vector.tensor_scalar(
                            out=posf[:, 0:1], in0=ef[:, 0:1],
                            scalar1=float(CAP), scalar2=None, op0=OP.mult)
                        nc.vector.tensor_scalar(
                            out=posf[:, 1:2], in0=ef[:, 1:2],
                            scalar1=float(CAP), scalar2=None, op0=OP.mult)
                        nc.vector.scalar_tensor_tensor(
                            out=posf[:, 0:1], in0=r0[:], scalar=-1.0,
                            in1=posf[:, 0:1], op0=OP.add, op1=OP.add)
                        nc.vector.scalar_tensor_tensor(
                            out=posf[:, 1:2], in0=r1[:], scalar=-1.0,
                            in1=posf[:, 1:2], op0=OP.add, op1=OP.add)
                        ovf = p0.tile([P, 2], dt.uint8, tag="ovf")
                        nc.vector.tensor_scalar(
                            out=ovf[:, 0:1], in0=r0[:], scalar1=float(CAP),
                            scalar2=None, op0=OP.is_gt)
                        nc.vector.tensor_scalar(
                            out=ovf[:, 1:2], in0=r1[:], scalar1=float(CAP),
                            scalar2=None, op0=OP.is_gt)
                        trash = p0.tile([P, 2], dt.float32, tag="trash")
                        nc.vector.memset(trash[:], float(TRASH))
                        nc.vector.copy_predicated(posf[:], ovf[:], trash[:])
                        pos_i = pp.tile([P, 2], dt.int32, tag=f"pos{i}")
                        nc.vector.tensor_copy(pos_i[:], posf[:])
                        pos_tiles.append(pos_i)

                        tok16 = p0.tile([P, 1], dt.int16, tag="tok16")
                        nc.sync.dma_start(tok16[:], iotat_ap[tsl, :])
                        for s in range(2):
                            nc.gpsimd.indirect_dma_start(
                                out=bid_dram[:, :],
                                out_offset=IndirectOffsetOnAxis(
                                    ap=pos_i[:, s:s + 1], axis=0),
                                in_=tok16[:, :], in_offset=None)

            # ---- dispatch gather (gpsimd/DMA) overlaps shared expert (PE) ----
            with tc.tile_pool(name="gx_pool", bufs=1) as pgx:
                gxT = pgx.tile([P, NIDX // P, ND, P], dt.bfloat16)
                idxw = pgx.tile([P, NIDX // 16], dt.int16)
                for g in range(8):
                    nc.sync.dma_start(
                        idxw[g * 16:(g + 1) * 16, :],
                        bid_dram[:NIDX, :].rearrange(
                            "(c p) one -> p (c one)", p=16))
                for j in range(NIDX // P):
                    nc.gpsimd.dma_gather(
                        out_ap=gxT[:, j, :, :],
                        in_ap=xbf_dram[:, :],
                        idxs_ap=idxw[:, j * 8:(j + 1) * 8],
                        num_idxs=P, num_idxs_reg=P, elem_size=D,
                        transpose=True)

                # -- shared expert --
                with (
                    tc.tile_pool(name="sw1p", bufs=1) as sw1p,
                    tc.tile_pool(name="sw2p", bufs=1) as sw2p,
                    tc.tile_pool(name="hsTp", bufs=1) as hsTp,
                    tc.tile_pool(name="ps3", bufs=2, space="PSUM") as ps3,
                    tc.tile_pool(name="ps4", bufs=2, space="PSUM") as ps4,
                ):
                    hsT = hsTp.tile([P, NF2, T], dt.bfloat16)
                    for m5 in range(F2 // 512):
                        sw1m = sw1p.tile([P, ND, 512], dt.bfloat16, tag="sw1m")
                        nc.scalar.dma_start(sw1m[:], sw1_ap[m5])
                        for mm in range(4):
                            m = m5 * 4 + mm
                            for n in range(2):
                                pm = ps3.tile([P, 512], dt.float32, tag="pm3",
                                              space="PSUM")
                                for k in range(ND):
                                    nc.tensor.matmul(
                                        pm[:],
                                        lhsT=sw1m[:, k, mm * P:(mm + 1) * P],
                                        rhs=xTb[:, 4 * n:4 * n + 4, k, :],
                                        start=(k == 0), stop=(k == ND - 1))
                                nc.scalar.activation(
                                    hsT[:, m, n * 512:(n + 1) * 512], pm[:],
                                    FT.Gelu, bias=sb1g_sb[:, m:m + 1],
                                    scale=sg_sb[:, m:m + 1])

                    sw2_sb = sw2p.tile([P, NF2, D], dt.bfloat16)
                    nc.scalar.dma_start(
                        sw2_sb[:], sw2_ap.rearrange("(k p) d2 -> p k d2", p=P))
                    for j in range(NT):
                        jsl = slice(j * P, (j + 1) * P)
                        for n in range(2):
                            pyt = ps4.tile([P, 512], dt.float32, tag="py4",
                                           space="PSUM")
                            for k in range(NF2):
                                nc.tensor.matmul(
                                    pyt[:], lhsT=hsT[:, k, jsl],
                                    rhs=sw2_sb[:, k, n * 512:(n + 1) * 512],
                                    start=(k == 0), stop=False)
                            nc.tensor.matmul(
                                pyt[:], lhsT=onesb[:, :],
                                rhs=sb2_sb[:, n * 512:(n + 1) * 512],
                                start=False, stop=True)
                            # ys = sigmoid(shared_weight) * (fc2s + sb2)
                            nc.scalar.activation(
                                ys[:, j, n * 512:(n + 1) * 512], pyt[:],
                                FT.Copy, scale=sig_bc[:, 0:1])

                # ---- expert FFNs ----
                with (
                    tc.tile_pool(name="w1p", bufs=3) as w1p,
                    tc.tile_pool(name="w2p", bufs=3) as w2p,
                    tc.tile_pool(name="hTp", bufs=1) as hTp,
                    tc.tile_pool(name="ps1", bufs=2, space="PSUM") as ps1,
                    tc.tile_pool(name="ps2", bufs=1, space="PSUM") as ps2,
                ):
                  for e in range(E):
                    hT = hTp.tile([P, NF, CAP], dt.bfloat16, tag="hT")
                    # fc1 over 512-wide F chunks
                    for m5 in range(F // 512):
                        w1m = w1p.tile([P, ND, 512], dt.bfloat16, tag="w1m")
                        nc.scalar.dma_start(w1m[:], w1_ap[e, m5])
                        for mm in range(4):
                            m = m5 * 4 + mm
                            pm = ps1.tile([P, CAP], dt.float32, tag="pm",
                                          space="PSUM")
                            for (kind, c, a, pos, take) in _fc1_segs(e):
                                for k in range(ND):
                                    if kind == "full":
                                        rhs = gxT[:, c:c + a, k, :]
                                    else:
                                        rhs = gxT[:, c, k, a:a + take]
                                    nc.tensor.matmul(
                                        pm[:, pos:pos + take],
                                        lhsT=w1m[:, k, mm * P:(mm + 1) * P],
                                        rhs=rhs,
                                        start=(k == 0), stop=(k == ND - 1))
                            nc.scalar.activation(
                                hT[:, m, :], pm[:], FT.Gelu,
                                bias=b1g_sb[:, e, m:m + 1],
                                scale=gate_sb[:, e, m:m + 1])
                    # fc2: 6 psum tiles held across the k loop
                    pys = [
                        ps2.tile([P, 512], dt.float32, tag=f"py{j}_{n}",
                                 name=f"py_e{e}_{j}_{n}", space="PSUM")
                        for j, (ro, rn) in enumerate(_cap_tiles())
                        for n in range(2)
                    ]
                    for k in range(NF):
                        w2k = w2p.tile([P, D], dt.bfloat16, tag="w2k")
                        nc.scalar.dma_start(w2k[:], w2_ap[e, k * P:(k + 1) * P, :])
                        pi = 0
                        for (ro, rn) in _cap_tiles():
                            for n in range(2):
                                nc.tensor.matmul(
                                    pys[pi][:rn, :],
                                    lhsT=hT[:, k, ro:ro + rn],
                                    rhs=w2k[:, n * 512:(n + 1) * 512],
                                    start=(k == 0), stop=False)
                                pi += 1
                    pi = 0
                    for (ro, rn) in _cap_tiles():
                        for n in range(2):
                            nc.tensor.matmul(
                                pys[pi][:rn, :], lhsT=onesb[:, :rn],
                                rhs=b2_sb[:, e, n * 512:(n + 1) * 512],
                                start=False, stop=True)
                            yev = w2p.tile([P, 512], dt.float32, tag="yev",
                                           name=f"yev_{e}_{pi}")
                            nc.vector.tensor_copy(yev[:rn, :], pys[pi][:rn, :])
                            nc.sync.dma_start(
                                ybk_dram[e * CAP + ro:e * CAP + ro + rn,
                                         n * 512:(n + 1) * 512],
                                yev[:rn, :])
                            pi += 1

            # ---- combine + LayerNorm ----
            with tc.tile_pool(name="ph5", bufs=3) as p5:
                lng_bc = pp.tile([P, D], dt.float32)
                nc.sync.dma_start(lng_bc[:], lng_ap.to_broadcast([P, D]))
                lnb_bc = pp.tile([P, D], dt.float32)
                nc.sync.dma_start(lnb_bc[:], lnb_ap.to_broadcast([P, D]))
                for i in range(NT):
                    g0 = p5.tile([P, D], dt.float32, tag="g0")
                    g1 = p5.tile([P, D], dt.float32, tag="g1")
                    nc.gpsimd.indirect_dma_start(
                        out=g0[:], out_offset=None, in_=ybk_dram[:, :],
                        in_offset=IndirectOffsetOnAxis(
                            ap=pos_tiles[i][:, 0:1], axis=0))
                    nc.gpsimd.indirect_dma_start(
                        out=g1[:], out_offset=None, in_=ybk_dram[:, :],
                        in_offset=IndirectOffsetOnAxis(
                            ap=pos_tiles[i][:, 1:2], axis=0))
                    comb = p5.tile([P, D], dt.float32, tag="comb")
                    nc.vector.scalar_tensor_tensor(
                        out=comb[:], in0=g0[:], scalar=cw_tiles[i][:, 0:1],
                        in1=ys[:, i, :], op0=OP.mult, op1=OP.add)
                    nc.vector.scalar_tensor_tensor(
                        out=comb[:], in0=g1[:], scalar=cw_tiles[i][:, 1:2],
                        in1=comb[:], op0=OP.mult, op1=OP.add)
                    mu = p5.tile([P, 1], dt.float32, tag="mu")
                    nc.vector.reduce_sum(mu[:], comb[:], axis=AX.X)
                    nmu = p5.tile([P, 1], dt.float32, tag="nmu")
                    nc.vector.tensor_scalar_mul(nmu[:], mu[:], -1.0 / D)
                    yc = p5.tile([P, D], dt.float32, tag="yc")
                    nc.scalar.activation(yc[:], comb[:], FT.Identity,
                                         bias=nmu[:, 0:1])
                    sq = p5.tile([P, D], dt.float32, tag="sq")
                    varsum = p5.tile([P, 1], dt.float32, tag="varsum")
                    nc.scalar.activation(sq[:], yc[:], FT.Square,
                                         accum_out=varsum[:])
                    sd = p5.tile([P, 1], dt.float32, tag="sd")
                    nc.scalar.activation(sd[:], varsum[:], FT.Sqrt,
                                         scale=1.0 / D, bias=eps_t[:, 0:1])
                    rinv = p5.tile([P, 1], dt.float32, tag="rinv")
                    nc.vector.reciprocal(rinv[:], sd[:])
                    o1 = p5.tile([P, D], dt.float32, tag="o1")
                    nc.vector.scalar_tensor_tensor(
                        out=o1[:], in0=yc[:], scalar=rinv[:, 0:1],
                        in1=lng_bc[:], op0=OP.mult, op1=OP.mult)
                    nc.vector.tensor_add(o1[:], o1[:], lnb_bc[:])
                    nc.sync.dma_start(out_ap[i * P:(i + 1) * P, :], o1[:])

    nc.compile()
    return nc


def _consts():
    iota8 = np.tile(np.arange(8, dtype=np.float32), (P, 1))
    iotat = np.arange(T, dtype=np.int16).reshape(T, 1)
    tri = np.triu(np.ones((P, P), np.float32)).astype(ml_dtypes.bfloat16)
    ident = np.eye(P, dtype=np.float32)
    onesb = np.ones((1, P), dtype=ml_dtypes.bfloat16)
    onesf = np.ones((1, P), dtype=np.float32)
    idw_id = np.arange(T, dtype=np.int16).reshape(T // 16, 16).T.copy()
    return dict(iota8=iota8, iotat=iotat, tri=tri, ident=ident,
                onesb=onesb, onesf=onesf, idw_id=idw_id)



def _pack_w1(w1f):
    """[E, D, F] f32 -> [E, F//512, P, ND, 512] bf16 (fc1 SBUF tile layout)."""
    bf = ml_dtypes.bfloat16
    return np.ascontiguousarray(
        np.asarray(w1f, np.float32).astype(bf)
        .reshape(E, ND, P, F // 512, 512).transpose(0, 3, 2, 1, 4))


def _pack_sw1(sw1f):
    """[D, F2] f32 -> [F2//512, P, ND, 512] bf16."""
    bf = ml_dtypes.bfloat16
    return np.ascontiguousarray(
        np.asarray(sw1f, np.float32).astype(bf)
        .reshape(ND, P, F2 // 512, 512).transpose(2, 1, 0, 3))


def make_in_maps(inputs):
    """Build the 8 per-core input maps from the full problem inputs."""
    bf = ml_dtypes.bfloat16
    x = np.ascontiguousarray(
        np.asarray(inputs["hidden_states"], np.float32).reshape(-1, D))
    shared = dict(
        rw=np.asarray(inputs["router_w"], np.float32),
        rb=np.asarray(inputs["router_b"], np.float32).reshape(1, E),
        w1=_pack_w1(inputs["w1"]),
        w2=np.asarray(inputs["w2"], np.float32).astype(bf),
        b1=np.ascontiguousarray(np.asarray(inputs["b1"], np.float32)
                                .reshape(E, NF, P).transpose(2, 0, 1)),
        gate=np.ascontiguousarray(np.asarray(inputs["gate"], np.float32)
                                  .reshape(E, NF, P).transpose(2, 0, 1)),
        b2=np.asarray(inputs["b2"], np.float32).astype(bf),
        sw1=_pack_sw1(inputs["sw1"]),
        sb1=np.ascontiguousarray(np.asarray(inputs["sb1"], np.float32)
                                 .reshape(NF2, P).T),
        sgate=np.ascontiguousarray(np.asarray(inputs["sgate"], np.float32)
                                   .reshape(NF2, P).T),
        sw2=np.asarray(inputs["sw2"], np.float32).astype(bf),
        sb2=np.asarray(inputs["sb2"], np.float32).astype(bf).reshape(1, D),
        shw=np.asarray(inputs["shared_weight"], np.float32).reshape(1, 1),
        lng=np.asarray(inputs["ln_g"], np.float32).reshape(1, D),
        lnb=np.asarray(inputs["ln_b"], np.float32).reshape(1, D),
        **_consts(),
    )
    return [{"x": np.ascontiguousarray(x[c * T:(c + 1) * T]), **shared}
            for c in range(NCORES)]


def kernel(hidden_states, router_w, router_b, w1, b1, gate, w2, b2,
           sw1, sb1, sgate, sw2, sb2, shared_weight, ln_g, ln_b):
    global _PROGRAM
    if _PROGRAM is None:
        _PROGRAM = build_program()
    nc = _PROGRAM

    in_maps = make_in_maps(dict(
        hidden_states=hidden_states, router_w=router_w, router_b=router_b,
        w1=w1, b1=b1, gate=gate, w2=w2, b2=b2, sw1=sw1, sb1=sb1, sgate=sgate,
        sw2=sw2, sb2=sb2, shared_weight=shared_weight, ln_g=ln_g, ln_b=ln_b))
    res = run_bass_kernel_spmd(nc, in_maps, list(range(NCORES)))
    out = np.concatenate([res.results[c]["out"] for c in range(NCORES)], axis=0)
    return out.reshape(B, S, D).astype(np.float32)


if __name__ == "__main__":
    build_program()
    print("kernel program built OK")


# revision 20
# speedup vs baseline: 1.2036x; 1.0167x over previous
"""Trainium2 Bass kernel for nn_ExpertFFNEnsemble (MoE routing, 8 experts, top-2).

Strategy: data-parallel over tokens (8192 tokens -> 1024/core, 8 cores).
Per core, fully on device:
  router (fp32)  -> top-2 + combine weights (sigmoid of logit gap)
  compaction     -> per-expert capacity buckets via cumsum-ranks + indirect scatter
  dispatch       -> one dma_gather(transpose=True) of all bucket rows (bf16)
  expert FFNs    -> bf16 matmuls, fp32 PSUM, exact-gelu ACT epilogue
  shared expert  -> dense bf16 FFN over the core's 1024 tokens
  combine + LN   -> indirect row gathers of the two bucket outputs + LayerNorm
No cross-core communication; host only shards tokens / casts weights to bf16
and concatenates the per-core output slices.
"""

import sys

sys.path.insert(0, "/opt/trn_rl_repo")

import numpy as np
import ml_dtypes

import concourse.bass as bass
import concourse.mybir as mybir
import concourse.tile as tile
from concourse import bacc
from concourse.bass import IndirectOffsetOnAxis
from concourse.bass_utils import run_bass_kernel_spmd

P = 128
B, S, D, F = 4, 2048, 1024, 4096
F2 = F // 2
E = 8
NCORES = 8
T = (B * S) // NCORES           # 1024 tokens per core
NT = T // P                     # 8 token tiles
ND = D // P                     # 8 d-chunks
NF = F // P                     # 32 f-chunks
NF2 = F2 // P                   # 16 f2-chunks
CAP = 320                       # per-expert token capacity (2.5 x 128)
NIDX = E * CAP                  # 2560 bucket rows (%128 == 0)
TRASH = NIDX                    # overflow-redirect row
BID_ROWS = NIDX + P             # bucket-id rows incl. trash region
YBK_ROWS = NIDX + P             # fc2 output rows incl. trash region
LN_EPS = 1e-5
FT = mybir.ActivationFunctionType
dt = mybir.dt
AX = mybir.AxisListType
OP = mybir.AluOpType

_PROGRAM = None


def _fc1_segs(e):
    """Moving-operand segments of expert e's bucket in the chunk-major gxT.

    Returns (kind, chunk, a, pos, take): kind 'full' -> a = #chunks,
    kind 'part' -> a = in-chunk offset.
    """
    segs = []
    pos = 0
    start = e * CAP
    while pos < CAP:
        c, off = divmod(start + pos, P)
        if off == 0 and (CAP - pos) >= P:
            n = (CAP - pos) // P
            segs.append(("full", c, n, pos, n * P))
            pos += n * P
        else:
            take = min(P - off, CAP - pos)
            segs.append(("part", c, off, pos, take))
            pos += take
    return segs


def _cap_tiles():
    """Token tiles within one expert bucket: list of (row_offset, rows)."""
    out = []
    r = 0
    while r < CAP:
        n = min(P, CAP - r)
        out.append((r, n))
        r += n
    return out


def build_program():
    nc = bacc.Bacc("TRN2", target_bir_lowering=False, debug=False,
                   num_devices=NCORES)

    def din(name, shape, dtype):
        return nc.dram_tensor(name, list(shape), dtype, kind="ExternalInput").ap()

    x_ap = din("x", [T, D], dt.float32)
    rw_ap = din("rw", [D, E], dt.float32)
    rb_ap = din("rb", [1, E], dt.float32)
    w1_ap = din("w1", [E, F // 512, P, ND, 512], dt.bfloat16)
    w2_ap = din("w2", [E, F, D], dt.bfloat16)
    b1_ap = din("b1", [P, E, NF], dt.float32)
    gate_ap = din("gate", [P, E, NF], dt.float32)
    b2_ap = din("b2", [E, D], dt.bfloat16)
    sw1_ap = din("sw1", [F2 // 512, P, ND, 512], dt.bfloat16)
    sb1_ap = din("sb1", [P, NF2], dt.float32)
    sgate_ap = din("sgate", [P, NF2], dt.float32)
    sw2_ap = din("sw2", [F2, D], dt.bfloat16)
    sb2_ap = din("sb2", [1, D], dt.bfloat16)
    shw_ap = din("shw", [1, 1], dt.float32)
    lng_ap = din("lng", [1, D], dt.float32)
    lnb_ap = din("lnb", [1, D], dt.float32)
    # host-provided constants
    iota8_ap = din("iota8", [P, 8], dt.float32)
    iotat_ap = din("iotat", [T, 1], dt.int16)
    tri_ap = din("tri", [P, P], dt.bfloat16)
    ident_ap = din("ident", [P, P], dt.float32)
    onesb_ap = din("onesb", [1, P], dt.bfloat16)
    onesf_ap = din("onesf", [1, P], dt.float32)
    idw_id_ap = din("idw_id", [16, T // 16], dt.int16)  # wrapped identity idx

    out_ap = nc.dram_tensor("out", [T, D], dt.float32, kind="ExternalOutput").ap()

    xbf_dram = nc.dram_tensor("xbf_i", [T, D], dt.bfloat16).ap()
    bid_dram = nc.dram_tensor("bid_i", [BID_ROWS, 1], dt.int16).ap()
    ybk_dram = nc.dram_tensor("ybk_i", [YBK_ROWS, D], dt.float32).ap()

    with tile.TileContext(nc) as tc:
        with (
            tc.tile_pool(name="persist", bufs=1) as pp,
            tc.tile_pool(name="small", bufs=1) as sp,
        ):
            # ---- constants ----
            iota8 = pp.tile([P, 8], dt.float32)
            nc.sync.dma_start(iota8[:], iota8_ap[:])
            tri = pp.tile([P, P], dt.bfloat16)
            nc.sync.dma_start(tri[:], tri_ap[:])
            ident = pp.tile([P, P], dt.float32)
            nc.sync.dma_start(ident[:], ident_ap[:])
            onesb = pp.tile([1, P], dt.bfloat16)
            nc.sync.dma_start(onesb[:], onesb_ap[:])
            onesf = pp.tile([1, P], dt.float32)
            nc.sync.dma_start(onesf[:], onesf_ap[:])
            rw_sb = pp.tile([P, ND, E], dt.float32)
            nc.sync.dma_start(rw_sb[:], rw_ap.rearrange("(k p) e -> p k e", p=P))
            rb_sb = pp.tile([1, E], dt.float32)
            nc.sync.dma_start(rb_sb[:], rb_ap[:, :])
            eps_t = pp.tile([P, 1], dt.float32)
            nc.vector.memset(eps_t[:], LN_EPS)

            # gate / b1*gate per expert: [128, E, NF]
            gate_sb = pp.tile([P, E, NF], dt.float32)
            nc.sync.dma_start(gate_sb[:], gate_ap[:])
            b1_sb = pp.tile([P, E, NF], dt.float32)
            nc.sync.dma_start(b1_sb[:], b1_ap[:])
            b1g_sb = pp.tile([P, E, NF], dt.float32)
            nc.vector.tensor_mul(b1g_sb[:], b1_sb[:], gate_sb[:])
            b2_sb = pp.tile([1, E, D], dt.bfloat16)
            nc.sync.dma_start(b2_sb[:], b2_ap.rearrange("e d2 -> e d2")[None, :, :])
            sg_sb = pp.tile([P, NF2], dt.float32)
            nc.sync.dma_start(sg_sb[:], sgate_ap[:])
            sb1_sb = pp.tile([P, NF2], dt.float32)
            nc.sync.dma_start(sb1_sb[:], sb1_ap[:])
            sb1g_sb = pp.tile([P, NF2], dt.float32)
            nc.vector.tensor_mul(sb1g_sb[:], sb1_sb[:], sg_sb[:])
            sb2_sb = pp.tile([1, D], dt.bfloat16)
            nc.sync.dma_start(sb2_sb[:], sb2_ap[:, :])

            # zero tiles for pre-clearing internal DRAM
            zid = sp.tile([P, BID_ROWS // P], dt.int16, tag="zid")
            nc.vector.memset(zid[:], 0)
            nc.sync.dma_start(
                bid_dram.rearrange("(p c) one -> p (c one)", p=P), zid[:])
            ztrash = sp.tile([P, D // 2], dt.float32, tag="ztrash")
            nc.vector.memset(ztrash[:], 0.0)
            nc.sync.dma_start(ybk_dram[TRASH:TRASH + P, :D // 2], ztrash[:])
            nc.sync.dma_start(ybk_dram[TRASH:TRASH + P, D // 2:], ztrash[:])

            shw_sb = sp.tile([1, 1], dt.float32, tag="shw")
            nc.sync.dma_start(shw_sb[:], shw_ap[:, :])
            sig1 = sp.tile([1, 1], dt.float32, tag="sig1")
            nc.scalar.activation(sig1[:], shw_sb[:], FT.Sigmoid)
            sig_bc = pp.tile([P, 1], dt.float32)

            # per-token routing results, kept for the combine phase
            cw_tiles = []
            pos_tiles = []

            # xT bf16, chunk-major: [128, NT, ND, P]
            xTb = pp.tile([P, NT, ND, P], dt.bfloat16)
            # shared-expert output (scaled), kept until combine
            ys = pp.tile([P, NT, D], dt.float32)

            with (
                tc.tile_pool(name="ph0", bufs=2) as p0,
                tc.tile_pool(name="ph0ps", bufs=1, space="PSUM") as p0ps,
            ):
                # -- phase 0 + router, with xTf in its own pool scope --
                with tc.tile_pool(name="xTf_pool", bufs=1) as pxT:
                    psig = p0ps.tile([P, 8], dt.float32, tag="rtr", name="psig",
                                     space="PSUM", bufs=2)
                    nc.tensor.matmul(psig[:, 0:1], lhsT=onesf[:, :],
                                     rhs=sig1[:, :], start=True, stop=True)
                    nc.vector.tensor_copy(sig_bc[:], psig[:, 0:1])

                    xTf = pxT.tile([P, ND, T], dt.float32)
                    for i in range(NT):
                        xt = p0.tile([P, D], dt.float32, tag="xt")
                        nc.sync.dma_start(xt[:], x_ap[i * P:(i + 1) * P, :])
                        xb = p0.tile([P, D], dt.bfloat16, tag="xb")
                        nc.vector.tensor_copy(xb[:], xt[:])
                        nc.sync.dma_start(xbf_dram[i * P:(i + 1) * P, :], xb[:])
                        for k in range(ND):
                            ptr = p0ps.tile([P, P], dt.float32, tag="ptr",
                                            space="PSUM", bufs=2)
                            nc.tensor.transpose(
                                ptr[:], xt[:, k * P:(k + 1) * P], ident[:])
                            nc.vector.tensor_copy(
                                xTf[:, k, i * P:(i + 1) * P], ptr[:])

                    # xTb via identity dma_gather (bf16, transposed)
                    idw_id = p0.tile([P, T // 16], dt.int16, tag="idw_id")
                    for g in range(8):
                        nc.sync.dma_start(idw_id[g * 16:(g + 1) * 16, :],
                                          idw_id_ap[:, :])
                    for j in range(T // P):
                        nc.gpsimd.dma_gather(
                            out_ap=xTb[:, j, :, :],
                            in_ap=xbf_dram[:, :],
                            idxs_ap=idw_id[:, j * 8:(j + 1) * 8],
                            num_idxs=P, num_idxs_reg=P, elem_size=D,
                            transpose=True)

                    # ---- router + compaction ----
                    carry = pp.tile([E, 1], dt.float32)
                    nc.vector.memset(carry[:], 0.0)
                    rank_sb = pp.tile([E, T], dt.float32)

                    for i in range(NT):
                        tsl = slice(i * P, (i + 1) * P)
                        pl = p0ps.tile([P, 8], dt.float32, tag="rtr",
                                       name=f"pl{i}", space="PSUM", bufs=2)
                        for k in range(ND):
                            nc.tensor.matmul(pl[:], lhsT=xTf[:, k, tsl],
                                             rhs=rw_sb[:, k, :],
                                             start=(k == 0), stop=False)
                        nc.tensor.matmul(pl[:], lhsT=onesf[:, :], rhs=rb_sb[:, :],
                                         start=False, stop=True)
                        vals = p0.tile([P, 8], dt.float32, tag="vals")
                        idx = p0.tile([P, 8], dt.uint32, tag="idx")
                        lt = p0.tile([P, 8], dt.float32, tag="lt")
                        nc.vector.tensor_copy(lt[:], pl[:])
                        nc.vector.max_with_indices(vals[:], idx[:], lt[:])

                        d01 = p0.tile([P, 1], dt.float32, tag="d01")
                        nc.vector.tensor_sub(d01[:], vals[:, 0:1], vals[:, 1:2])
                        cw = pp.tile([P, 2], dt.float32, tag=f"cw{i}")
                        nc.scalar.activation(cw[:, 0:1], d01[:], FT.Sigmoid)
                        nc.scalar.activation(cw[:, 1:2], d01[:], FT.Sigmoid,
                                             scale=-1.0)
                        cw_tiles.append(cw)

                        ef = p0.tile([P, 2], dt.float32, tag="ef")
                        nc.vector.tensor_copy(ef[:], idx[:, 0:2])
                        oh0 = p0.tile([P, 8], dt.float32, tag="oh0")
                        oh1 = p0.tile([P, 8], dt.float32, tag="oh1")
                        nc.vector.tensor_tensor(
                            out=oh0[:], in0=ef[:, 0:1].to_broadcast([P, 8]),
                            in1=iota8[:], op=OP.is_equal)
                        nc.vector.tensor_tensor(
                            out=oh1[:], in0=ef[:, 1:2].to_broadcast([P, 8]),
                            in1=iota8[:], op=OP.is_equal)
                        A = p0.tile([P, 8], dt.bfloat16, tag="A")
                        nc.vector.tensor_add(A[:], oh0[:], oh1[:])

                        pr = p0ps.tile([E, P], dt.float32, tag="rtr",
                                       name=f"pr{i}", space="PSUM", bufs=2)
                        nc.tensor.matmul(pr[:], lhsT=A[:], rhs=tri[:],
                                         start=True, stop=True)
                        nc.vector.tensor_scalar_add(rank_sb[:, tsl], pr[:],
                                                    carry[:, 0:1])
                        nc.vector.tensor_copy(
                            carry[:], rank_sb[:, i * P + P - 1:i * P + P])

                        prt = p0ps.tile([P, E], dt.float32, tag="rtr",
                                        name=f"prt{i}", space="PSUM", bufs=2)
                        nc.tensor.transpose(prt[:], rank_sb[:, tsl],
                                            ident[:E, :E])
                        rank_t = p0.tile([P, E], dt.float32, tag="rank_t")
                        nc.vector.tensor_copy(rank_t[:], prt[:])

                        tmp = p0.tile([P, 8], dt.float32, tag="tmp")
                        r0 = p0.tile([P, 1], dt.float32, tag="r0")
                        r1 = p0.tile([P, 1], dt.float32, tag="r1")
                        nc.vector.tensor_mul(tmp[:], oh0[:], rank_t[:])
                        nc.vector.reduce_sum(r0[:], tmp[:], axis=AX.X)
                        nc.vector.tensor_mul(tmp[:], oh1[:], rank_t[:])
                        nc.vector.reduce_sum(r1[:], tmp[:], axis=AX.X)

                        posf = p0.tile([P, 2], dt.float32, tag="posf")
                        nc.vector.tensor_scalar(
                            out=posf[:, 0:1], in0=ef[:, 0:1],
                            scalar1=float(CAP), scalar2=None, op0=OP.mult)
                        nc.vector.tensor_scalar(
                            out=posf[:, 1:2], in0=ef[:, 1:2],
                            scalar1=float(CAP), scalar2=None, op0=OP.mult)
                        nc.vector.scalar_tensor_tensor(
                            out=posf[:, 0:1], in0=r0[:], scalar=-1.0,
                            in1=posf[:, 0:1], op0=OP.add, op1=OP.add)
                        nc.vector.scalar_tensor_tensor(
                            out=posf[:, 1:2], in0=r1[:], scalar=-1.0,
                            in1=posf[:, 1:2], op0=OP.add, op1=OP.add)
                        ovf = p0.tile([P, 2], dt.uint8, tag="ovf")
                        nc.vector.tensor_scalar(
                            out=ovf[:, 0:1], in0=r0[:], scalar1=float(CAP),
                            scalar2=None, op0=OP.is_gt)
                        nc.vector.tensor_scalar(
                            out=ovf[:, 1:2], in0=r1[:], scalar1=float(CAP),
                            scalar2=None, op0=OP.is_gt)
                        trash = p0.tile([P, 2], dt.float32, tag="trash")
                        nc.vector.memset(trash[:], float(TRASH))
                        nc.vector.copy_predicated(posf[:], ovf[:], trash[:])
                        pos_i = pp.tile([P, 2], dt.int32, tag=f"pos{i}")
                        nc.vector.tensor_copy(pos_i[:], posf[:])
                        pos_tiles.append(pos_i)

                        tok16 = p0.tile([P, 1], dt.int16, tag="tok16")
                        nc.sync.dma_start(tok16[:], iotat_ap[tsl, :])
                        for s in range(2):
                            nc.gpsimd.indirect_dma_start(
                                out=bid_dram[:, :],
                                out_offset=IndirectOffsetOnAxis(
                                    ap=pos_i[:, s:s + 1], axis=0),
                                in_=tok16[:, :], in_offset=None)

            # ---- dispatch gather (gpsimd/DMA) overlaps shared expert (PE) ----
            with tc.tile_pool(name="gx_pool", bufs=1) as pgx:
                gxT = pgx.tile([P, NIDX // P, ND, P], dt.bfloat16)
                idxw = pgx.tile([P, NIDX // 16], dt.int16)
                for g in range(8):
                    nc.sync.dma_start(
                        idxw[g * 16:(g + 1) * 16, :],
                        bid_dram[:NIDX, :].rearrange(
                            "(c p) one -> p (c one)", p=16))
                for j in range(NIDX // P):
                    nc.gpsimd.dma_gather(
                        out_ap=gxT[:, j, :, :],
                        in_ap=xbf_dram[:, :],
                        idxs_ap=idxw[:, j * 8:(j + 1) * 8],
                        num_idxs=P, num_idxs_reg=P, elem_size=D,
                        transpose=True)

                # -- shared expert --
                with (
                    tc.tile_pool(name="sw1p", bufs=2) as sw1p,
                    tc.tile_pool(name="sw2p", bufs=1) as sw2p,
                    tc.tile_pool(name="hsTp", bufs=1) as hsTp,
                    tc.tile_pool(name="ps3", bufs=2, space="PSUM") as ps3,
                    tc.tile_pool(name="ps4", bufs=2, space="PSUM") as ps4,
                ):
                    hsT = hsTp.tile([P, NF2, T], dt.bfloat16)
                    for m5 in range(F2 // 512):
                        sw1m = sw1p.tile([P, ND, 512], dt.bfloat16, tag="sw1m")
                        nc.scalar.dma_start(sw1m[:], sw1_ap[m5])
                        for mm in range(4):
                            m = m5 * 4 + mm
                            for n in range(2):
                                pm = ps3.tile([P, 512], dt.float32, tag="pm3",
                                              space="PSUM")
                                for k in range(ND):
                                    nc.tensor.matmul(
                                        pm[:],
                                        lhsT=sw1m[:, k, mm * P:(mm + 1) * P],
                                        rhs=xTb[:, 4 * n:4 * n + 4, k, :],
                                        start=(k == 0), stop=(k == ND - 1))
                                nc.scalar.activation(
                                    hsT[:, m, n * 512:(n + 1) * 512], pm[:],
                                    FT.Gelu, bias=sb1g_sb[:, m:m + 1],
                                    scale=sg_sb[:, m:m + 1])

                    sw2_sb = sw2p.tile([P, NF2, D], dt.bfloat16)
                    nc.scalar.dma_start(
                        sw2_sb[:], sw2_ap.rearrange("(k p) d2 -> p k d2", p=P))
                    for j in range(NT):
                        jsl = slice(j * P, (j + 1) * P)
                        for n in range(2):
                            pyt = ps4.tile([P, 512], dt.float32, tag="py4",
                                           space="PSUM")
                            for k in range(NF2):
                                nc.tensor.matmul(
                                    pyt[:], lhsT=hsT[:, k, jsl],
                                    rhs=sw2_sb[:, k, n * 512:(n + 1) * 512],
                                    start=(k == 0), stop=False)
                            nc.tensor.matmul(
                                pyt[:], lhsT=onesb[:, :],
                                rhs=sb2_sb[:, n * 512:(n + 1) * 512],
                                start=False, stop=True)
                            # ys = sigmoid(shared_weight) * (fc2s + sb2)
                            nc.scalar.activation(
                                ys[:, j, n * 512:(n + 1) * 512], pyt[:],
                                FT.Copy, scale=sig_bc[:, 0:1])

                # ---- expert FFNs ----
                with (
                    tc.tile_pool(name="w1p", bufs=3) as w1p,
                    tc.tile_pool(name="w2p", bufs=3) as w2p,
                    tc.tile_pool(name="hTp", bufs=1) as hTp,
                    tc.tile_pool(name="ps1", bufs=2, space="PSUM") as ps1,
                    tc.tile_pool(name="ps2", bufs=1, space="PSUM") as ps2,
                ):
                  for e in range(E):
                    hT = hTp.tile([P, NF, CAP], dt.bfloat16, tag="hT")
                    # fc1 over 512-wide F chunks
                    for m5 in range(F // 512):
                        w1m = w1p.tile([P, ND, 512], dt.bfloat16, tag="w1m")
                        nc.scalar.dma_start(w1m[:], w1_ap[e, m5])
                        for mm in range(4):
                            m = m5 * 4 + mm
                            pm = ps1.tile([P, CAP], dt.float32, tag="pm",
                                          space="PSUM")
                            for (kind, c, a, pos, take) in _fc1_segs(e):
                                for k in range(ND):
                                    if kind == "full":
                                        rhs = gxT[:, c:c + a, k, :]
                                    else:
                                        rhs = gxT[:, c, k, a:a + take]
                                    nc.tensor.matmul(
                                        pm[:, pos:pos + take],
                                        lhsT=w1m[:, k, mm * P:(mm + 1) * P],
                                        rhs=rhs,
                                        start=(k == 0), stop=(k == ND - 1))
                            nc.scalar.activation(
                                hT[:, m, :], pm[:], FT.Gelu,
                                bias=b1g_sb[:, e, m:m + 1],
                                scale=gate_sb[:, e, m:m + 1])
                    # fc2: 6 psum tiles held across the k loop
                    pys = [
                        ps2.tile([P, 512], dt.float32, tag=f"py{j}_{n}",
                                 name=f"py_e{e}_{j}_{n}", space="PSUM")
                        for j, (ro, rn) in enumerate(_cap_tiles())
                        for n in range(2)
                    ]
                    for k in range(NF):
                        w2k = w2p.tile([P, D], dt.bfloat16, tag="w2k")
                        nc.scalar.dma_start(w2k[:], w2_ap[e, k * P:(k + 1) * P, :])
                        pi = 0
                        for (ro, rn) in _cap_tiles():
                            for n in range(2):
                                nc.tensor.matmul(
                                    pys[pi][:rn, :],
                                    lhsT=hT[:, k, ro:ro + rn],
                                    rhs=w2k[:, n * 512:(n + 1) * 512],
                                    start=(k == 0), stop=False)
                                pi += 1
                    pi = 0
                    for (ro, rn) in _cap_tiles():
                        for n in range(2):
                            nc.tensor.matmul(
                                pys[pi][:rn, :], lhsT=onesb[:, :rn],
                                rhs=b2_sb[:, e, n * 512:(n + 1) * 512],
                                start=False, stop=True)
                            yev = w2p.tile([P, 512], dt.float32, tag="yev",
                                           name=f"yev_{e}_{pi}")
                            nc.vector.tensor_copy(yev[:rn, :], pys[pi][:rn, :])
                            nc.sync.dma_start(
                                ybk_dram[e * CAP + ro:e * CAP + ro + rn,
                                         n * 512:(n + 1) * 512],
                                yev[:rn, :])
                            pi += 1

            # ---- combine + LayerNorm ----
            with (
                tc.tile_pool(name="ph5", bufs=3) as p5,
                tc.tile_pool(name="ph5g", bufs=8) as p5g,
            ):
                lng_bc = pp.tile([P, D], dt.float32)
                nc.sync.dma_start(lng_bc[:], lng_ap.to_broadcast([P, D]))
                lnb_bc = pp.tile([P, D], dt.float32)
                nc.sync.dma_start(lnb_bc[:], lnb_ap.to_broadcast([P, D]))
                for i in range(NT):
                    g0 = p5g.tile([P, D], dt.float32, tag="g0")
                    g1 = p5g.tile([P, D], dt.float32, tag="g1")
                    nc.gpsimd.indirect_dma_start(
                        out=g0[:], out_offset=None, in_=ybk_dram[:, :],
                        in_offset=IndirectOffsetOnAxis(
                            ap=pos_tiles[i][:, 0:1], axis=0))
                    nc.gpsimd.indirect_dma_start(
                        out=g1[:], out_offset=None, in_=ybk_dram[:, :],
                        in_offset=IndirectOffsetOnAxis(
                            ap=pos_tiles[i][:, 1:2], axis=0))
                    comb = p5.tile([P, D], dt.float32, tag="comb")
                    nc.vector.scalar_tensor_tensor(
                        out=comb[:], in0=g0[:], scalar=cw_tiles[i][:, 0:1],
                        in1=ys[:, i, :], op0=OP.mult, op1=OP.add)
                    nc.vector.scalar_tensor_tensor(
                        out=comb[:], in0=g1[:], scalar=cw_tiles[i][:, 1:2],
                        in1=comb[:], op0=OP.mult, op1=OP.add)
                    mu = p5.tile([P, 1], dt.float32, tag="mu")
                    nc.vector.reduce_sum(mu[:], comb[:], axis=AX.X)
                    nmu = p5.tile([P, 1], dt.float32, tag="nmu")
                    nc.vector.tensor_scalar_mul(nmu[:], mu[:], -1.0 / D)
                    yc = p5.tile([P, D], dt.float32, tag="yc")
                    nc.scalar.activation(yc[:], comb[:], FT.Identity,
                                         bias=nmu[:, 0:1])
                    sq = p5.tile([P, D], dt.float32, tag="sq")
                    varsum = p5.tile([P, 1], dt.float32, tag="varsum")
                    nc.scalar.activation(sq[:], yc[:], FT.Square,
                                         accum_out=varsum[:])
                    sd = p5.tile([P, 1], dt.float32, tag="sd")
                    nc.scalar.activation(sd[:], varsum[:], FT.Sqrt,
                                         scale=1.0 / D, bias=eps_t[:, 0:1])
                    rinv = p5.tile([P, 1], dt.float32, tag="rinv")
                    nc.vector.reciprocal(rinv[:], sd[:])
                    o1 = p5.tile([P, D], dt.float32, tag="o1")
                    nc.vector.scalar_tensor_tensor(
                        out=o1[:], in0=yc[:], scalar=rinv[:, 0:1],
                        in1=lng_bc[:], op0=OP.mult, op1=OP.mult)
                    nc.vector.tensor_add(o1[:], o1[:], lnb_bc[:])
                    nc.sync.dma_start(out_ap[i * P:(i + 1) * P, :], o1[:])

    nc.compile()
    return nc


def _consts():
    iota8 = np.tile(np.arange(8, dtype=np.float32), (P, 1))
    iotat = np.arange(T, dtype=np.int16).reshape(T, 1)
    tri = np.triu(np.ones((P, P), np.float32)).astype(ml_dtypes.bfloat16)
    ident = np.eye(P, dtype=np.float32)
    onesb = np.ones((1, P), dtype=ml_dtypes.bfloat16)
    onesf = np.ones((1, P), dtype=np.float32)
    idw_id = np.arange(T, dtype=np.int16).reshape(T // 16, 16).T.copy()
    return dict(iota8=iota8, iotat=iotat, tri=tri, ident=ident,
                onesb=onesb, onesf=onesf, idw_id=idw_id)



def _pack_w1(w1f):
    """[E, D, F] f32 -> [E, F//512, P, ND, 512] bf16 (fc1 SBUF tile layout)."""
    bf = ml_dtypes.bfloat16
    return np.ascontiguousarray(
        np.asarray(w1f, np.float32).astype(bf)
        .reshape(E, ND, P, F // 512, 512).transpose(0, 3, 2, 1, 4))


def _pack_sw1(sw1f):
    """[D, F2] f32 -> [F2//512, P, ND, 512] bf16."""
    bf = ml_dtypes.bfloat16
    return np.ascontiguousarray(
        np.asarray(sw1f, np.float32).astype(bf)
        .reshape(ND, P, F2 // 512, 512).transpose(2, 1, 0, 3))


def make_in_maps(inputs):
    """Build the 8 per-core input maps from the full problem inputs."""
    bf = ml_dtypes.bfloat16
    x = np.ascontiguousarray(
        np.asarray(inputs["hidden_states"], np.float32).reshape(-1, D))
    shared = dict(
        rw=np.asarray(inputs["router_w"], np.float32),
        rb=np.asarray(inputs["router_b"], np.float32).reshape(1, E),
        w1=_pack_w1(inputs["w1"]),
        w2=np.asarray(inputs["w2"], np.float32).astype(bf),
        b1=np.ascontiguousarray(np.asarray(inputs["b1"], np.float32)
                                .reshape(E, NF, P).transpose(2, 0, 1)),
        gate=np.ascontiguousarray(np.asarray(inputs["gate"], np.float32)
                                  .reshape(E, NF, P).transpose(2, 0, 1)),
        b2=np.asarray(inputs["b2"], np.float32).astype(bf),
        sw1=_pack_sw1(inputs["sw1"]),
        sb1=np.ascontiguousarray(np.asarray(inputs["sb1"], np.float32)
                                 .reshape(NF2, P).T),
        sgate=np.ascontiguousarray(np.asarray(inputs["sgate"], np.float32)
                                   .reshape(NF2, P).T),
        sw2=np.asarray(inputs["sw2"], np.float32).astype(bf),
        sb2=np.asarray(inputs["sb2"], np.float32).astype(bf).reshape(1, D),
        shw=np.asarray(inputs["shared_weight"], np.float32).reshape(1, 1),
        lng=np.asarray(inputs["ln_g"], np.float32).reshape(1, D),
        lnb=np.asarray(inputs["ln_b"], np.float32).reshape(1, D),
        **_consts(),
    )
    return [{"x": np.ascontiguousarray(x[c * T:(c + 1) * T]), **shared}
            for c in range(NCORES)]


def kernel(hidden_states, router_w, router_b, w1, b1, gate, w2, b2,
           sw1, sb1, sgate, sw2, sb2, shared_weight, ln_g, ln_b):
    global _PROGRAM
    if _PROGRAM is None:
        _PROGRAM = build_program()
    nc = _PROGRAM

    in_maps = make_in_maps(dict(
        hidden_states=hidden_states, router_w=router_w, router_b=router_b,
        w1=w1, b1=b1, gate=gate, w2=w2, b2=b2, sw1=sw1, sb1=sb1, sgate=sgate,
        sw2=sw2, sb2=sb2, shared_weight=shared_weight, ln_g=ln_g, ln_b=ln_b))
    res = run_bass_kernel_spmd(nc, in_maps, list(range(NCORES)))
    out = np.concatenate([res.results[c]["out"] for c in range(NCORES)], axis=0)
    return out.reshape(B, S, D).astype(np.float32)


if __name__ == "__main__":
    build_program()
    print("kernel program built OK")
